# revision 1
# baseline (speedup 1.0000x reference)
"""MLA (multi-head latent attention) Bass kernel for 8 trn2 NeuronCores.

Sharding: core = b*4 + g  (b in {0,1} batches, g in {0..3} head-groups of 4 heads).
Each core computes, for its batch b and 4 heads:
  - projections in feature-major ("transposed") layout from xT (float32r matmuls),
  - flash-style causal attention with scores computed k-major (S^T) so the
    exp'd probabilities feed the PV matmul directly (no transposes),
  - LOBO softmax: attn = exp(s) / (sum_k exp(s) + C*exp(max_k s)); the row max
    is recovered as max_k exp(s) via a DMA max-accumulate (CCE) into a per-head
    comb tile + a DVE 32x32-transpose reduction,
  - row-parallel output projection -> partial [T, E] f32.
Host sums the 4 partials per batch (the all-reduce of the row-parallel design).
"""

import math
import os

import numpy as np

import concourse.bass as bass
import concourse.mybir as mybir
import concourse.tile as _tile_mod
from concourse.tile import TileContext
from concourse.vector_clock import ScopedClock, VectorClock
import bass_rust as _bass_rust
from concourse.bass_utils import run_bass_kernel_spmd

_N_PROCS = _bass_rust.N_PROCS


def _split_drain_and_barrier(self, tick_clock, wait_clock):
    """Replacement for TileContext._drain_and_barrier: the stock version puts
    the whole global vector clock (up to 27 sem waits) on one Drain, which this
    walrus rejects ("Too many sync wait commands").  Emit one Drain per
    outstanding processor instead."""
    gc = tick_clock.global_clock
    procs = [p for p in range(_N_PROCS) if gc[p] > 0]
    for p in procs:
        vc = VectorClock([gc[q] if q == p else 0 for q in range(_N_PROCS)])
        d = self.nc.sync.drain()
        wait_clock.add_sem_waits(d.ins, ScopedClock({None: vc}))
    self.nc.all_engine_barrier()
    popped = self.nc._tile_sem_poison_stack.pop()
    assert popped is self._sem_poison
    self.nc.clear_and_free_semaphores(list(self.sems.allocated().values()))
    self.nc.all_engine_barrier()


_tile_mod.TileContext._drain_and_barrier = _split_drain_and_barrier

# ---------------------------------------------------------------------------
# This walrus build enforces small per-instruction sync-wait budgets
# ("Too many sync wait commands").  Post-process the BIR JSON: any
# instruction carrying more than its budget of waits gets the excess
# hoisted onto same-engine Drain carriers inserted immediately before it
# (same program point on the engine's sequential stream -> semantics
# unchanged).
# ---------------------------------------------------------------------------
_orig_to_json_bytes = bass.Bass.to_json_bytes
_WAIT_LIMITS = {"Drain": 1, "DMACopy": 1}
_DEF_WAIT_LIMIT = 1


def _to_json_split_waits(self, *a, **kw):
    import json as _json
    data = _json.loads(_orig_to_json_bytes(self, *a, **kw))
    nid = 0
    for f in data.get("functions", []):
        for bb in f.get("blocks", []):
            out = []
            for inst in bb.get("instructions", []):
                si = inst.get("sync_info")
                if isinstance(si, dict):
                    w = si.get("on_wait")
                    if isinstance(w, list):
                        k = _WAIT_LIMITS.get(inst.get("opcode"), _DEF_WAIT_LIMIT)
                        if len(w) > k:
                            extra, keep = w[:-k], w[-k:]
                            for wt in extra:
                                out.append({
                                    "debug": inst.get("debug"),
                                    "engine": inst["engine"],
                                    "ins": [], "outs": [],
                                    "name": f"wsplit-{nid}",
                                    "opcode": "Drain",
                                    "sync_info": {"on_update": [],
                                                  "on_wait": [wt]},
                                })
                                nid += 1
                            si["on_wait"] = keep
                out.append(inst)
            bb["instructions"] = out
    return _json.dumps(data).encode()


bass.Bass.to_json_bytes = _to_json_split_waits

B, T, E = 2, 2048, 1024
H, DH = 16, 64
DKV = 256
DR = 32
HL = 4              # heads per core
NG = 4              # head groups
SCALE = 1.0 / math.sqrt(DH + DR)
TG = 512            # query-group width
KC = 128            # key-chunk width
NTG = T // TG       # 4
NKC = T // KC       # 16
EC = E // 128       # 8  e-chunks
CC = DKV // 128     # 2  latent chunks

F32 = mybir.dt.float32
F32R = mybir.dt.float32r
BF16 = mybir.dt.bfloat16
AF = mybir.ActivationFunctionType
ALU = mybir.AluOpType
AX = mybir.AxisListType

_CACHE = {}


def _r(ap):
    return ap.bitcast(F32R)


def _build_program():
    nc = bass.Bass()

    xT = nc.declare_dram_parameter("xT", [E, T], F32, isOutput=False)
    wq = nc.declare_dram_parameter("wq", [E, HL * DH], F32, isOutput=False)
    wqr = nc.declare_dram_parameter("wqr", [E, HL * DR], F32, isOutput=False)
    wkr = nc.declare_dram_parameter("wkr", [E, DR], F32, isOutput=False)
    wkvd = nc.declare_dram_parameter("wkvd", [E, DKV], F32, isOutput=False)
    wku = nc.declare_dram_parameter("wku", [DKV, HL * DH], F32, isOutput=False)
    wvu = nc.declare_dram_parameter("wvu", [DKV, HL * DH], F32, isOutput=False)
    wo = nc.declare_dram_parameter("wo", [HL * DH, E], F32, isOutput=False)
    cosq = nc.declare_dram_parameter("cosq", [HL * DR, T], F32, isOutput=False)
    sinq = nc.declare_dram_parameter("sinq", [HL * DR, T], F32, isOutput=False)
    lobo = nc.declare_dram_parameter("lobo", [HL, 1], F32, isOutput=False)
    masks = nc.declare_dram_parameter("masks", [128, 4 * TG], F32, isOutput=False)
    out = nc.declare_dram_parameter("out", [T, E], F32, isOutput=True)

    with TileContext(nc) as tc:
        from contextlib import ExitStack

        with ExitStack() as ctx:
            singles = ctx.enter_context(tc.tile_pool(name="singles", bufs=1))
            pool = ctx.enter_context(tc.tile_pool(name="pool", bufs=2))
            psp = ctx.enter_context(tc.tile_pool(name="psp", bufs=1, space="PSUM"))

            # ---------------- weights (f32; x-side used as f32r) ----------------
            wq_sb = singles.tile([128, EC, HL * DH], BF16)
            nc.gpsimd.dma_start(
                out=wq_sb, in_=wq.rearrange("(c p) f -> p c f", p=128))
            wqr_sb = singles.tile([128, EC, HL * DR], BF16)
            nc.gpsimd.dma_start(
                out=wqr_sb, in_=wqr.rearrange("(c p) f -> p c f", p=128))
            wkr_sb = singles.tile([128, EC, DR], BF16)
            nc.gpsimd.dma_start(
                out=wkr_sb, in_=wkr.rearrange("(c p) f -> p c f", p=128))
            wkvd_sb = singles.tile([128, EC, DKV], BF16)
            nc.gpsimd.dma_start(
                out=wkvd_sb, in_=wkvd.rearrange("(c p) f -> p c f", p=128))
            # latent-side weights in bf16 (latT is bf16)
            wku_sb = singles.tile([128, CC, HL * DH], BF16)
            nc.gpsimd.dma_start(
                out=wku_sb, in_=wku.rearrange("(c p) f -> p c f", p=128))
            wvu_sb = singles.tile([128, CC, HL * DH], BF16)
            nc.gpsimd.dma_start(
                out=wvu_sb, in_=wvu.rearrange("(c p) f -> p c f", p=128))
            wo_sb = singles.tile([128, 2, E], BF16)
            nc.gpsimd.dma_start(
                out=wo_sb, in_=wo.rearrange("(c p) e -> p c e", p=128))

            cosq_sb = singles.tile([128, T], BF16)
            nc.gpsimd.dma_start(out=cosq_sb, in_=cosq[:, :])
            sinq_sb = singles.tile([128, T], BF16)
            nc.gpsimd.dma_start(out=sinq_sb, in_=sinq[:, :])
            lobo_sb = singles.tile([HL, 1], F32)
            nc.sync.dma_start(out=lobo_sb, in_=lobo[:, :])
            c_sb = singles.tile([HL, 1], F32)
            nc.scalar.activation(c_sb, lobo_sb, AF.Exp)

            # causal masks for the 4 diagonal offsets: keep iff x - y - 128*j <= 0
            masks_sb = singles.tile([128, 4, TG], BF16)
            nc.gpsimd.dma_start(
                out=masks_sb, in_=masks.rearrange("p (j y) -> p j y", j=4))

            ones_sb = singles.tile([1, DH], F32)
            nc.vector.memset(ones_sb, 1.0)

            # ---------------- persistent activation tiles ----------------
            latT_sb = singles.tile([128, CC, T], BF16)
            qT = [singles.tile([96, T], BF16, name=f"qT{h}") for h in range(HL)]
            kT = [singles.tile([96, T], BF16, name=f"kT{h}") for h in range(HL)]
            rp_pre = singles.tile([128, T], BF16)
            rp_swap = singles.tile([128, T], BF16)
            rp_m1 = singles.tile([128, T], BF16)
            rp_m2 = singles.tile([128, T], BF16)
            kr_pre = singles.tile([DR, T], BF16)
            xt_sb = singles.tile([128, EC, T], BF16)
            nc.gpsimd.dma_start(
                out=xt_sb, in_=xT.rearrange("(c p) t -> p c t", p=128))

            # ---------------- projections from xT, streamed per tg ----------------
            for tg in range(NTG):
                ts = slice(tg * TG, (tg + 1) * TG)
                xts = [xt_sb[:, ec, ts] for ec in range(EC)]
                # latent halves + k_rope
                pa = psp.tile([128, TG], F32, name="pa", tag="A", bufs=3)
                pb = psp.tile([128, TG], F32, name="pb", tag="B", bufs=3)
                pc = psp.tile([128, TG], F32, name="pc", tag="C", bufs=2)
                for ec in range(EC):
                    nc.tensor.matmul(
                        pa, (wkvd_sb[:, ec, 0:128]), (xts[ec]),
                        start=(ec == 0), stop=(ec == EC - 1))
                    nc.tensor.matmul(
                        pb, (wkvd_sb[:, ec, 128:256]), (xts[ec]),
                        start=(ec == 0), stop=(ec == EC - 1))
                    nc.tensor.matmul(
                        pc[0:DR, :], (wkr_sb[:, ec, :]), (xts[ec]),
                        start=(ec == 0), stop=(ec == EC - 1))
                nc.vector.tensor_copy(latT_sb[:, 0, ts], pa)
                nc.vector.tensor_copy(latT_sb[:, 1, ts], pb)
                nc.scalar.copy(kr_pre[:, ts], pc[0:DR, :])
                # q projections
                pa = psp.tile([128, TG], F32, name="pa", tag="A", bufs=3)
                pb = psp.tile([128, TG], F32, name="pb", tag="B", bufs=3)
                pc = psp.tile([128, TG], F32, name="pc", tag="C", bufs=2)
                for ec in range(EC):
                    nc.tensor.matmul(
                        pa, (wq_sb[:, ec, 0:128]), (xts[ec]),
                        start=(ec == 0), stop=(ec == EC - 1))
                    nc.tensor.matmul(
                        pb, (wq_sb[:, ec, 128:256]), (xts[ec]),
                        start=(ec == 0), stop=(ec == EC - 1))
                    nc.tensor.matmul(
                        pc, (wqr_sb[:, ec, :]), (xts[ec]),
                        start=(ec == 0), stop=(ec == EC - 1))
                st = pool.tile([128, TG], BF16, name="st0", tag="qkstage", bufs=3)
                nc.scalar.copy(st, pa)
                nc.sync.dma_start(out=qT[0][0:DH, ts], in_=st[0:DH, :])
                nc.sync.dma_start(out=qT[1][0:DH, ts], in_=st[DH:128, :])
                st = pool.tile([128, TG], BF16, name="st1", tag="qkstage", bufs=3)
                nc.scalar.copy(st, pb)
                nc.sync.dma_start(out=qT[2][0:DH, ts], in_=st[0:DH, :])
                nc.sync.dma_start(out=qT[3][0:DH, ts], in_=st[DH:128, :])
                nc.scalar.copy(rp_pre[:, ts], pc)
                # rope on q_r rows for this tg
                for h in range(HL):
                    nc.sync.dma_start(
                        out=rp_swap[h * DR:h * DR + 16, ts],
                        in_=rp_pre[h * DR + 16:h * DR + 32, ts])
                    nc.sync.dma_start(
                        out=rp_swap[h * DR + 16:h * DR + 32, ts],
                        in_=rp_pre[h * DR:h * DR + 16, ts])
                nc.vector.tensor_mul(rp_m1[:, ts], rp_pre[:, ts], cosq_sb[:, ts])
                nc.vector.tensor_mul(rp_m2[:, ts], rp_swap[:, ts], sinq_sb[:, ts])
                nc.vector.tensor_add(rp_m2[:, ts], rp_m1[:, ts], rp_m2[:, ts])
                for h in range(HL):
                    nc.sync.dma_start(
                        out=qT[h][DH:96, ts], in_=rp_m2[h * DR:(h + 1) * DR, ts])
                # rope on k_r rows for this tg
                nc.sync.dma_start(
                    out=rp_swap[0:16, ts], in_=kr_pre[16:32, ts])
                nc.sync.dma_start(
                    out=rp_swap[16:32, ts], in_=kr_pre[0:16, ts])
                nc.vector.tensor_mul(
                    rp_m1[0:DR, ts], kr_pre[:, ts], cosq_sb[0:DR, ts])
                nc.vector.tensor_mul(
                    rp_m2[0:DR, ts], rp_swap[0:DR, ts], sinq_sb[0:DR, ts])
                nc.vector.tensor_add(
                    rp_m2[0:DR, ts], rp_m1[0:DR, ts], rp_m2[0:DR, ts])
                for h in range(HL):
                    nc.sync.dma_start(out=kT[h][DH:96, ts], in_=rp_m2[0:DR, ts])

            # ---------------- k_c from latentT ----------------
            for hp in range(2):
                for tg in range(NTG):
                    ts = slice(tg * TG, (tg + 1) * TG)
                    pa = psp.tile([128, TG], F32, name="pa", tag="A", bufs=3)
                    for cc in range(CC):
                        nc.tensor.matmul(
                            pa, wku_sb[:, cc, hp * 128:(hp + 1) * 128],
                            latT_sb[:, cc, ts],
                            start=(cc == 0), stop=(cc == CC - 1))
                    st = pool.tile([128, TG], BF16, name="st2", tag="qkstage", bufs=3)
                    nc.vector.tensor_copy(st, pa)
                    nc.sync.dma_start(out=kT[2 * hp][0:DH, ts], in_=st[0:DH, :])
                    nc.sync.dma_start(
                        out=kT[2 * hp + 1][0:DH, ts], in_=st[DH:128, :])

            # ---------------- V (natural layout, +ones column) ----------------
            v_sb = singles.tile([128, NKC, HL, DH + 1], BF16)
            nc.vector.memset(v_sb, 1.0)
            for tt in range(NKC):
                pb = psp.tile([128, HL * DH], F32, name="pv", tag="B", bufs=3)
                for cc in range(CC):
                    nc.tensor.matmul(
                        pb, latT_sb[:, cc, tt * 128:(tt + 1) * 128],
                        wvu_sb[:, cc, :],
                        start=(cc == 0), stop=(cc == CC - 1))
                nc.vector.tensor_copy(v_sb[:, tt, :, 0:DH], pb)

            # ---------------- attention ----------------
            yraw_sb = singles.tile([DH, HL, T], BF16)
            dsum_sb = singles.tile([HL, T], F32)
            emax_sb = singles.tile([HL, T], F32)
            emst_sb = singles.tile([HL, T], F32)

            for h in range(HL):
                comb = pool.tile([128, T], BF16, name="comb", tag="comb", bufs=1)
                nc.vector.memset(comb, 0.0)
                for qg in range(NTG):
                    qs = slice(qg * TG, (qg + 1) * TG)
                    nkc = 4 * qg + 4
                    yps = psp.tile([DH + 1, TG], F32, name="py", tag="B", bufs=3)
                    for kc in range(nkc):
                        sps = psp.tile([128, TG], F32, name="ps", tag="A", bufs=3)
                        nc.tensor.matmul(
                            sps, kT[h][:, kc * KC:(kc + 1) * KC], qT[h][:, qs])
                        pt = pool.tile(
                            [128, TG], BF16, name="pt", tag="ptile", bufs=4)
                        nc.scalar.activation(pt, sps, AF.Exp, scale=SCALE)
                        j = kc - 4 * qg
                        if j >= 0:
                            nc.gpsimd.tensor_mul(pt, pt, masks_sb[:, j, :])
                        nc.vector.tensor_max(comb[:, qs], comb[:, qs], pt)
                        nc.tensor.matmul(
                            yps, v_sb[:, kc, h, :], pt,
                            start=(kc == 0), stop=(kc == nkc - 1))
                    nc.scalar.copy(yraw_sb[:, h, qs], yps[0:DH, :])
                    std = pool.tile([DH + 1, TG], F32, name="std", tag="stgd", bufs=1)
                    nc.scalar.copy(std[DH:DH + 1, :], yps[DH:DH + 1, :])
                    nc.sync.dma_start(
                        out=dsum_sb[h:h + 1, qs], in_=std[DH:DH + 1, :])
                # emax for this head: partition-max of comb via 32x32 transpose
                combT = pool.tile([128, T], BF16, name="combT", tag="combT", bufs=1)
                nc.vector.transpose(combT, comb)
                red = pool.tile([128, T // 32], F32, name="red", tag="red", bufs=1)
                nc.vector.reduce_max(
                    red, combT.rearrange("p (b j) -> p b j", j=32), axis=AX.X)
                stk = pool.tile([32, 4, T // 32], F32, name="stk", tag="stk", bufs=1)
                for a in range(4):
                    nc.sync.dma_start(
                        out=stk[:, a, :], in_=red[a * 32:(a + 1) * 32, :])
                emf = pool.tile([32, T // 32], F32, name="emf", tag="emf", bufs=1)
                nc.vector.reduce_max(
                    emf, stk.rearrange("p a b -> p b a"), axis=AX.X)
                nc.sync.dma_start(out=emst_sb[h:h + 1, :], in_=emf)

            # ---------------- denominators + normalize ----------------
            # un-permute the per-head maxes (i-major -> natural q order)
            nc.vector.tensor_copy(
                emax_sb.rearrange("p (b i) -> p i b", i=32),
                emst_sb.rearrange("p (i b) -> p i b", b=64))
            # d = dsum + C * emax  (in place into dsum), r = 1/d (into emax)
            nc.vector.scalar_tensor_tensor(
                out=dsum_sb, in0=emax_sb, scalar=c_sb, in1=dsum_sb,
                op0=ALU.mult, op1=ALU.add)
            nc.vector.reciprocal(emax_sb, dsum_sb)

            yT_sb = singles.tile([128, 2, T], BF16)
            for h in range(HL):
                for qg in range(NTG):
                    qs = slice(qg * TG, (qg + 1) * TG)
                    rhh = pool.tile([1, TG], F32, name="rh", tag="rh", bufs=2)
                    nc.sync.dma_start(out=rhh, in_=emax_sb[h:h + 1, qs])
                    bc = psp.tile([DH, TG], F32, name="bc", tag="C", bufs=2)
                    nc.tensor.matmul(bc, ones_sb, rhh)
                    yn = pool.tile([DH, TG], BF16, name="yn", tag="yn", bufs=3)
                    nc.vector.tensor_mul(yn, yraw_sb[:, h, qs], bc)
                    nc.sync.dma_start(
                        out=yT_sb[(h % 2) * DH:(h % 2 + 1) * DH, h // 2, qs],
                        in_=yn)

            # ---------------- output projection (row-parallel partial) ----------------
            for tt in range(NKC):
                for eg in range(2):
                    pa = psp.tile([128, TG], F32, name="po", tag="A", bufs=3)
                    for fc in range(2):
                        nc.tensor.matmul(
                            pa, yT_sb[:, fc, tt * 128:(tt + 1) * 128],
                            wo_sb[:, fc, eg * TG:(eg + 1) * TG],
                            start=(fc == 0), stop=(fc == 1))
                    ost = pool.tile([128, TG], F32, name="ost", tag="ost", bufs=2)
                    if (tt + eg) % 2 == 0:
                        nc.scalar.copy(ost, pa)
                    else:
                        nc.vector.tensor_copy(ost, pa)
                    nc.sync.dma_start(
                        out=out[tt * 128:(tt + 1) * 128, eg * TG:(eg + 1) * TG],
                        in_=ost)

    return nc


def _masks():
    x = np.arange(128)[:, None]
    y = np.arange(TG)[None, :]
    ms = [(x - y + 128 * j <= 0).astype(np.float32) for j in range(4)]
    return np.concatenate(ms, axis=1)  # [128, 4*TG]


def _rope_tables():
    half = DR // 2
    inv = 1.0 / (10000.0 ** (np.arange(half, dtype=np.float64) / half))
    ang = np.arange(T, dtype=np.float64)[:, None] * inv[None, :]  # (T, half)
    cos = np.cos(ang).T  # (half, T)
    sin = np.sin(ang).T
    cosk = np.concatenate([cos, cos], axis=0)                 # (32, T)
    sink = np.concatenate([-sin, sin], axis=0)
    cosq = np.tile(cosk, (HL, 1)).astype(np.float32)          # (128, T)
    sinq = np.tile(sink, (HL, 1)).astype(np.float32)
    return cosq, sinq


def kernel(x, Wq, Wqr, Wkr, Wkvd, Wku, Wvu, Wo, lobo_log):
    x = np.asarray(x, dtype=np.float32)
    Wq = np.asarray(Wq, dtype=np.float32)
    Wqr = np.asarray(Wqr, dtype=np.float32)
    Wkr = np.asarray(Wkr, dtype=np.float32)
    Wkvd = np.asarray(Wkvd, dtype=np.float32)
    Wku = np.asarray(Wku, dtype=np.float32)
    Wvu = np.asarray(Wvu, dtype=np.float32)
    Wo = np.asarray(Wo, dtype=np.float32)
    lobo_log = np.asarray(lobo_log, dtype=np.float32)

    if "nc" not in _CACHE:
        _CACHE["nc"] = _build_program()
    nc = _CACHE["nc"]

    cosq, sinq = _rope_tables()
    msk = _masks()
    in_maps = []
    for core in range(8):
        b, g = core // NG, core % NG
        hs = slice(g * HL * DH, (g + 1) * HL * DH)
        rs = slice(g * HL * DR, (g + 1) * HL * DR)
        in_maps.append({
            "xT": np.ascontiguousarray(x[b].T),
            "wq": np.ascontiguousarray(Wq[:, hs]),
            "wqr": np.ascontiguousarray(Wqr[:, rs]),
            "wkr": Wkr,
            "wkvd": Wkvd,
            "wku": np.ascontiguousarray(Wku[:, hs]),
            "wvu": np.ascontiguousarray(Wvu[:, hs]),
            "wo": np.ascontiguousarray(Wo[hs, :]),
            "cosq": cosq, "sinq": sinq, "masks": msk,
            "lobo": np.ascontiguousarray(
                lobo_log[g * HL:(g + 1) * HL].reshape(HL, 1)),
        })

    trace = bool(os.environ.get("BASS_TRACE_KERNEL"))
    bkr = run_bass_kernel_spmd(
        nc, in_maps, core_ids=list(range(8)), trace=trace)
    if trace:
        print(f"HW exec time: {bkr.exec_time_ns} ns")
        if bkr.instructions_and_trace is not None:
            print("trace:", bkr.instructions_and_trace[1])
        _CACHE["last_result"] = bkr
    res = bkr.results
    out = np.zeros((B, T, E), dtype=np.float32)
    for core in range(8):
        out[core // NG] += res[core]["out"]
    return out



# revision 14
# speedup vs baseline: 1.5444x; 1.5444x over previous
"""MLA (multi-head latent attention) Bass kernel for 8 trn2 NeuronCores — v2.

Sharding: core = b*4 + g  (b in {0,1} batches, g in {0..3} head-groups of 4 heads).

v2 redesign vs v1 (461us):
  - all inputs pre-cast to bf16 on HOST (halves load bytes; sync-ring HWDGE,
    no gpsimd cast DMAs), x loaded in per-(tg,ec) 128KB chunks so matmuls
    start early.
  - projections pack ONE head per 128-row PSUM slot: [q_c(64)|q_rope(32)|spare(32)]
    with Wkr riding head0's spare rows; rope half-swap via DVE stream_shuffle
    (32-lane permute) + TT ops reading PSUM directly -> zero SBUF<->SBUF
    staging DMAs.
  - attention: kc chunks processed in PAIRS sharing a [128,1024] 2-bank PSUM
    tile; ONE scalar exp per pair (scalar engine is the attention bottleneck).
    Causal masks split DVE/gpsimd. LOBO row-max tracked per (h,qg) in a comb
    tile; partition-reduce via DVE 32x32 transpose + shuffle-based 4-group
    max tree, all in a (j,b)-permuted [32,16] layout; denominator + reciprocal
    on [32,16] tiles; tiny perm<->natural DMAs on the idle sync ring.
  - normalize (bc broadcast matmul + DVE mul) software-pipelined one qg stage
    behind attention; output projection for head3's completed query blocks
    interleaves with head3's attention; out stored as bf16 partials
    (host all-reduces in f32).
"""

import math
import os

import numpy as np
import ml_dtypes

import concourse.bass as bass
import concourse.mybir as mybir
import concourse.tile as _tile_mod
from concourse.tile import TileContext
from concourse.vector_clock import ScopedClock, VectorClock
import bass_rust as _bass_rust
from concourse.bass_utils import run_bass_kernel_spmd

_N_PROCS = _bass_rust.N_PROCS
BF = ml_dtypes.bfloat16


def _split_drain_and_barrier(self, tick_clock, wait_clock):
    """Replacement for TileContext._drain_and_barrier: the stock version puts
    the whole global vector clock (up to 27 sem waits) on one Drain, which this
    walrus rejects ("Too many sync wait commands").  Emit one Drain per
    outstanding processor instead."""
    gc = tick_clock.global_clock
    procs = [p for p in range(_N_PROCS) if gc[p] > 0]
    for p in procs:
        vc = VectorClock([gc[q] if q == p else 0 for q in range(_N_PROCS)])
        d = self.nc.sync.drain()
        wait_clock.add_sem_waits(d.ins, ScopedClock({None: vc}))
    self.nc.all_engine_barrier()
    popped = self.nc._tile_sem_poison_stack.pop()
    assert popped is self._sem_poison
    self.nc.clear_and_free_semaphores(list(self.sems.allocated().values()))
    self.nc.all_engine_barrier()


_tile_mod.TileContext._drain_and_barrier = _split_drain_and_barrier

# ---------------------------------------------------------------------------
# This walrus build enforces small per-instruction sync-wait budgets
# ("Too many sync wait commands").  Post-process the BIR JSON: any
# instruction carrying more than its budget of waits gets the excess
# hoisted onto same-engine Drain carriers inserted immediately before it
# (same program point on the engine's sequential stream -> semantics
# unchanged).
# ---------------------------------------------------------------------------
_orig_to_json_bytes = bass.Bass.to_json_bytes
_WAIT_LIMITS = {"Drain": 1, "DMACopy": 1}
_DEF_WAIT_LIMIT = 1


def _to_json_split_waits(self, *a, **kw):
    import json as _json
    data = _json.loads(_orig_to_json_bytes(self, *a, **kw))
    nid = 0
    for f in data.get("functions", []):
        for bb in f.get("blocks", []):
            out = []
            for inst in bb.get("instructions", []):
                si = inst.get("sync_info")
                if isinstance(si, dict):
                    w = si.get("on_wait")
                    if isinstance(w, list):
                        k = _WAIT_LIMITS.get(inst.get("opcode"), _DEF_WAIT_LIMIT)
                        if len(w) > k:
                            extra, keep = w[:-k], w[-k:]
                            for wt in extra:
                                out.append({
                                    "debug": inst.get("debug"),
                                    "engine": inst["engine"],
                                    "ins": [], "outs": [],
                                    "name": f"wsplit-{nid}",
                                    "opcode": "Drain",
                                    "sync_info": {"on_update": [],
                                                  "on_wait": [wt]},
                                })
                                nid += 1
                            si["on_wait"] = keep
                out.append(inst)
            bb["instructions"] = out
    return _json.dumps(data).encode()


bass.Bass.to_json_bytes = _to_json_split_waits

B, T, E = 2, 2048, 1024
H, DH = 16, 64
DKV = 256
DR = 32
HL = 4              # heads per core
NG = 4              # head groups
SCALE = 1.0 / math.sqrt(DH + DR)
TG = 512            # query-group width
KC = 128            # key-chunk width
NTG = T // TG       # 4
NKC = T // KC       # 16
EC = E // 128       # 8  e-chunks
CC = DKV // 128     # 2  latent chunks

F32 = mybir.dt.float32
BF16 = mybir.dt.bfloat16
AF = mybir.ActivationFunctionType
ALU = mybir.AluOpType
AX = mybir.AxisListType

_CACHE = {}

_ROT16 = [(i + 16) % 32 for i in range(32)]
_IDENT = list(range(32))


def _build_program():
    nc = bass.Bass()

    xT = nc.declare_dram_parameter("xT", [E, T], BF16, isOutput=False)
    wq = nc.declare_dram_parameter("wq", [E, HL * 128], BF16, isOutput=False)
    wkvd = nc.declare_dram_parameter("wkvd", [E, DKV], BF16, isOutput=False)
    wku = nc.declare_dram_parameter("wku", [DKV, HL * DH], BF16, isOutput=False)
    wvu = nc.declare_dram_parameter("wvu", [DKV, HL * DH], BF16, isOutput=False)
    wo = nc.declare_dram_parameter("wo", [HL * DH, E], BF16, isOutput=False)
    cosq = nc.declare_dram_parameter("cosq", [DR, T], BF16, isOutput=False)
    sinq = nc.declare_dram_parameter("sinq", [DR, T], BF16, isOutput=False)
    cvals = nc.declare_dram_parameter("cvals", [32, HL], F32, isOutput=False)
    masks = nc.declare_dram_parameter("masks", [128, 4 * TG], BF16, isOutput=False)
    ident = nc.declare_dram_parameter("ident", [128, 128], BF16, isOutput=False)
    out = nc.declare_dram_parameter("out", [T, E], BF16, isOutput=True)

    xTr = xT.rearrange("(c p) t -> p c t", p=128)

    with TileContext(nc) as tc:
        from contextlib import ExitStack

        with ExitStack() as ctx:
            singles = ctx.enter_context(tc.tile_pool(name="singles", bufs=1))
            pool = ctx.enter_context(tc.tile_pool(name="pool", bufs=2))
            psp = ctx.enter_context(tc.tile_pool(name="psp", bufs=1, space="PSUM"))

            # ---------------- weights + tables (all bf16 from host) --------
            wq_sb = singles.tile([128, EC, HL * 128], BF16)
            nc.sync.dma_start(out=wq_sb, in_=wq.rearrange("(c p) f -> p c f", p=128))
            wkvd_sb = singles.tile([128, EC, DKV], BF16)
            nc.sync.dma_start(
                out=wkvd_sb, in_=wkvd.rearrange("(c p) f -> p c f", p=128))
            cos_sb = singles.tile([DR, T], BF16)
            nc.sync.dma_start(out=cos_sb, in_=cosq[:, :])
            sin_sb = singles.tile([DR, T], BF16)
            nc.sync.dma_start(out=sin_sb, in_=sinq[:, :])
            wku_sb = singles.tile([128, CC, HL * DH], BF16)
            nc.sync.dma_start(out=wku_sb, in_=wku.rearrange("(c p) f -> p c f", p=128))
            wvu_sb = singles.tile([128, CC, HL * DH], BF16)
            nc.sync.dma_start(out=wvu_sb, in_=wvu.rearrange("(c p) f -> p c f", p=128))

            # x chunks for tg0 before the late-phase weights
            xt_sb = singles.tile([128, EC, T], BF16)
            for ec in range(EC):
                nc.sync.dma_start(out=xt_sb[:, ec, 0:TG], in_=xTr[:, ec, 0:TG])

            wo_sb = singles.tile([128, 2, E], BF16)
            nc.sync.dma_start(out=wo_sb, in_=wo.rearrange("(c p) e -> p c e", p=128))
            masks_sb = singles.tile([128, 4, TG], BF16)
            nc.sync.dma_start(
                out=masks_sb, in_=masks.rearrange("p (j y) -> p j y", j=4))
            ident_sb = singles.tile([128, 128], BF16)
            nc.sync.dma_start(out=ident_sb, in_=ident[:, :])
            onerow = singles.tile([1, TG], BF16)
            nc.vector.memset(onerow, 1.0)
            dext = singles.tile([32, TG], F32)
            nc.vector.memset(dext, 1.0)
            rstage = singles.tile([32, TG], BF16)
            nc.vector.memset(rstage, 1.0)
            c32_sb = singles.tile([32, HL], F32)
            nc.sync.dma_start(out=c32_sb, in_=cvals[:, :])

            for tg in range(1, NTG):
                ts = slice(tg * TG, (tg + 1) * TG)
                for ec in range(EC):
                    nc.sync.dma_start(out=xt_sb[:, ec, ts], in_=xTr[:, ec, ts])

            ones1 = singles.tile([1, DH], BF16)
            nc.vector.memset(ones1, 1.0)

            # ---------------- persistent activation tiles ----------------
            latT_sb = singles.tile([128, CC, T], BF16)
            qT = [singles.tile([96, T], BF16, name=f"qT{h}") for h in range(HL)]
            kT = [singles.tile([96, T], BF16, name=f"kT{h}") for h in range(HL)]
            v_sb = singles.tile([128, NKC, HL, DH + 1], BF16)
            nc.vector.memset(v_sb, 1.0)
            yraw_sb = singles.tile([DH, HL, T], BF16)
            yT_sb = singles.tile([128, 2, T], BF16)

            # ---------------- projections, per query-group ----------------
            for tg in range(NTG):
                ts = slice(tg * TG, (tg + 1) * TG)
                xts = [xt_sb[:, ec, ts] for ec in range(EC)]

                # q slots: two [128,1024] psum tiles, one head per 512-col half
                qps = []
                for hp in range(2):
                    sp = psp.tile([128, 2 * TG], F32, name=f"q{hp}", tag="S", bufs=2)
                    for hh in range(2):
                        h = 2 * hp + hh
                        fs = slice(h * 128, (h + 1) * 128)
                        for ec in range(EC):
                            nc.tensor.matmul(
                                sp[:, hh * TG:(hh + 1) * TG],
                                wq_sb[:, ec, fs], xts[ec],
                                start=(ec == 0), stop=(ec == EC - 1))
                    qps.append(sp)
                # latent: one [128,1024] psum tile (halves = latent rows 0:128/128:256)
                lp = psp.tile([128, 2 * TG], F32, name="lat", tag="S", bufs=2)
                for cc in range(CC):
                    for ec in range(EC):
                        nc.tensor.matmul(
                            lp[:, cc * TG:(cc + 1) * TG],
                            wkvd_sb[:, ec, cc * 128:(cc + 1) * 128], xts[ec],
                            start=(ec == 0), stop=(ec == EC - 1))

                # consumers: q_c copies (scalar), rope (DVE), latent (DVE)
                for h in range(HL):
                    sp = qps[h // 2]
                    hs = slice((h % 2) * TG, (h % 2 + 1) * TG)
                    nc.scalar.copy(qT[h][0:DH, ts], sp[0:DH, hs])

                def rope(dst, pre_ap):
                    sw = pool.tile([DR, TG], F32, name="sw", tag="qsw", bufs=2)
                    nc.vector.stream_shuffle(sw, pre_ap, _ROT16)
                    t1 = pool.tile([DR, TG], BF16, name="t1", tag="t1", bufs=2)
                    nc.vector.tensor_mul(t1, pre_ap, cos_sb[:, ts])
                    t2 = pool.tile([DR, TG], BF16, name="t2", tag="t2", bufs=2)
                    nc.vector.tensor_mul(t2, sw, sin_sb[:, ts])
                    nc.vector.tensor_add(dst, t1, t2)

                # head0 rope, then k_rope (both read qps[0] -> release it early)
                rope(qT[0][DH:96, ts], qps[0][DH:DH + DR, 0:TG])
                rope(kT[0][DH:96, ts], qps[0][96:128, 0:TG])
                for h in range(1, HL):
                    sp = qps[h // 2]
                    hs = slice((h % 2) * TG, (h % 2 + 1) * TG)
                    rope(qT[h][DH:96, ts], sp[DH:DH + DR, hs])
                for h in range(1, HL):
                    nc.scalar.copy(kT[h][DH:96, ts], kT[0][DH:96, ts])
                # latent -> SBUF bf16
                nc.vector.tensor_copy(latT_sb[:, 0, ts], lp[:, 0:TG])
                nc.vector.tensor_copy(latT_sb[:, 1, ts], lp[:, TG:2 * TG])

                # k_c = Wku^T latent (per head-pair), consumed by scalar copies
                for hp in range(2):
                    mp = psp.tile([128, TG], F32, name="kc", tag="M", bufs=2)
                    for cc in range(CC):
                        nc.tensor.matmul(
                            mp, wku_sb[:, cc, hp * 128:(hp + 1) * 128],
                            latT_sb[:, cc, ts],
                            start=(cc == 0), stop=(cc == CC - 1))
                    nc.scalar.copy(kT[2 * hp][0:DH, ts], mp[0:DH, :])
                    nc.scalar.copy(kT[2 * hp + 1][0:DH, ts], mp[DH:128, :])

                # V natural layout
                for tt in range(4 * tg, 4 * tg + 4):
                    bp = psp.tile([128, HL * DH], F32, name="v", tag="B", bufs=2)
                    for cc in range(CC):
                        nc.tensor.matmul(
                            bp, latT_sb[:, cc, tt * 128:(tt + 1) * 128],
                            wvu_sb[:, cc, :],
                            start=(cc == 0), stop=(cc == CC - 1))
                    nc.vector.tensor_copy(v_sb[:, tt, :, 0:DH], bp)

            # ---------------- attention ----------------
            # bc+yn for stage (h,qg) is emitted one iteration later so the
            # epilogue's DVE chain never stalls the tensor queue; head3's
            # finished query blocks trigger the output projection inline.
            norm_queue = []

            def emit_norm(h, qg, rback):
                qs = slice(qg * TG, (qg + 1) * TG)
                bcp = psp.tile([DH, TG], F32, name="bc", tag="B", bufs=2)
                nc.tensor.matmul(bcp, ones1, rback[0:1, :])
                nc.vector.tensor_mul(
                    yT_sb[(h % 2) * DH:(h % 2 + 1) * DH, h // 2, qs],
                    yraw_sb[0:DH, h, qs], bcp)

            def emit_outproj(qg):
                for tt in range(4 * qg, 4 * qg + 4):
                    op = psp.tile([128, E], F32, name="op", tag="S", bufs=2)
                    for eg in range(2):
                        for fc in range(2):
                            nc.tensor.matmul(
                                op[:, eg * TG:(eg + 1) * TG],
                                yT_sb[:, fc, tt * 128:(tt + 1) * 128],
                                wo_sb[:, fc, eg * TG:(eg + 1) * TG],
                                start=(fc == 0), stop=(fc == 1))
                    ost = pool.tile([128, E], BF16, name="ost", tag="ost", bufs=2)
                    nc.vector.tensor_copy(ost, op)
                    nc.sync.dma_start(
                        out=out[tt * 128:(tt + 1) * 128, :], in_=ost)

            for h in range(HL):
                for qg in range(NTG):
                    qs = slice(qg * TG, (qg + 1) * TG)
                    nkc = 4 * qg + 4
                    npair = nkc // 2
                    yps = psp.tile([DH + 1, TG], F32, name="py", tag="M", bufs=2)
                    comb = pool.tile([128, TG], BF16, name="comb", tag="comb",
                                     bufs=2)
                    for pr in range(npair):
                        kca, kcb = 2 * pr, 2 * pr + 1
                        diag = 4 * qg  # first diagonal chunk
                        sp = psp.tile([128, 2 * TG], F32, name="ps", tag="S",
                                      bufs=2)
                        for hv, kc in ((0, kca), (1, kcb)):
                            half = sp[:, hv * TG:(hv + 1) * TG]
                            j = kc - diag
                            nc.tensor.matmul(
                                half, kT[h][:, kc * KC:(kc + 1) * KC],
                                qT[h][:, qs],
                                start=True, stop=(j < 0))
                            if j >= 0:
                                # additive causal mask: += ident^T @ (-1e4 pattern)
                                w = 128 * j + 128
                                nc.tensor.matmul(
                                    sp[:, hv * TG:hv * TG + w],
                                    ident_sb, masks_sb[:, j, 0:w],
                                    start=False, stop=True)
                        pt = pool.tile([128, 2 * TG], BF16, name="pt", tag="pt",
                                       bufs=3)
                        nc.scalar.activation(pt, sp, AF.Exp)
                        if pr == 0:
                            nc.vector.tensor_max(comb, pt[:, 0:TG], pt[:, TG:2 * TG])
                        else:
                            nc.vector.tensor_max(comb, comb, pt[:, 0:TG])
                            nc.vector.tensor_max(comb, comb, pt[:, TG:2 * TG])
                        nc.tensor.matmul(
                            yps, v_sb[:, kca, h, :], pt[:, 0:TG],
                            start=(pr == 0), stop=False)
                        nc.tensor.matmul(
                            yps, v_sb[:, kcb, h, :], pt[:, TG:2 * TG],
                            start=False, stop=(pr == npair - 1))
                    # epilogue for (h, qg): emax via transpose+shuffle tree (perm
                    # (j,b) layout), d + reciprocal via transpose round-trip.
                    nc.vector.tensor_copy(yraw_sb[:, h, qs], yps[0:DH, :])
                    nc.vector.tensor_mul(dext[0:1, :], yps[DH:DH + 1, :], onerow)
                    combT = pool.tile([128, TG], BF16, name="combT", tag="combT",
                                      bufs=2)
                    nc.vector.transpose(combT, comb)
                    red = pool.tile([128, 16], F32, name="red", tag="red", bufs=2)
                    nc.vector.reduce_max(
                        red, combT.rearrange("p (b j) -> p b j", j=32), axis=AX.X)
                    s1 = pool.tile([32, 16], F32, name="s1", tag="s1", bufs=2)
                    nc.vector.stream_shuffle(s1, red[32:64, :], _IDENT)
                    s2 = pool.tile([32, 16], F32, name="s2", tag="s2", bufs=2)
                    nc.vector.stream_shuffle(s2, red[64:96, :], _IDENT)
                    s3 = pool.tile([32, 16], F32, name="s3", tag="s3", bufs=2)
                    nc.vector.stream_shuffle(s3, red[96:128, :], _IDENT)
                    nc.vector.tensor_max(s1, red[0:32, :], s1)
                    nc.vector.tensor_max(s2, s2, s3)
                    emfP = pool.tile([32, 16], F32, name="emf", tag="emf", bufs=2)
                    nc.vector.tensor_max(emfP, s1, s2)
                    dTt = pool.tile([32, TG], F32, name="dTt", tag="dTt", bufs=2)
                    nc.vector.transpose(dTt, dext)
                    dP = pool.tile([32, 16], F32, name="dP", tag="dP", bufs=2)
                    nc.vector.scalar_tensor_tensor(
                        out=dP, in0=emfP, scalar=c32_sb[:, h:h + 1],
                        in1=dTt.rearrange("p (b j) -> p b j", j=32)[:, :, 0],
                        op0=ALU.mult, op1=ALU.add)
                    with nc.allow_low_precision(reason="r in bf16; tol 2e-2"):
                        nc.vector.reciprocal(
                            rstage.rearrange("p (b j) -> p b j", j=32)[:, :, 0], dP)
                    rback = pool.tile([32, TG], BF16, name="rbk", tag="rbk", bufs=2)
                    nc.vector.transpose(rback, rstage)
                    # queue normalization; drain one stage behind
                    norm_queue.append((h, qg, rback))
                    if len(norm_queue) > 1:
                        hq, qq, r_ = norm_queue.pop(0)
                        emit_norm(hq, qq, r_)
                        if hq == 3:
                            emit_outproj(qq)
            while norm_queue:
                hq, qq, r_ = norm_queue.pop(0)
                emit_norm(hq, qq, r_)
                if hq == 3:
                    emit_outproj(qq)

    return nc


def _masks():
    """Additive causal masks: -1e4 on masked (k > q) cells, 0 elsewhere."""
    x = np.arange(128)[:, None]
    y = np.arange(TG)[None, :]
    ms = [np.where(x - y + 128 * j <= 0, 0.0, -1e4).astype(np.float32)
          for j in range(4)]
    return np.concatenate(ms, axis=1)  # [128, 4*TG]


def _rope_tables():
    half = DR // 2
    inv = 1.0 / (10000.0 ** (np.arange(half, dtype=np.float64) / half))
    ang = np.arange(T, dtype=np.float64)[:, None] * inv[None, :]  # (T, half)
    cos = np.cos(ang).T  # (half, T)
    sin = np.sin(ang).T
    cosk = np.concatenate([cos, cos], axis=0)                 # (32, T)
    sink = np.concatenate([-sin, sin], axis=0)
    return cosk.astype(np.float32), sink.astype(np.float32)


def _bf(x):
    return np.ascontiguousarray(np.asarray(x, dtype=np.float32).astype(BF))


def kernel(x, Wq, Wqr, Wkr, Wkvd, Wku, Wvu, Wo, lobo_log):
    x = np.asarray(x, dtype=np.float32)
    Wq = np.asarray(Wq, dtype=np.float32)
    Wqr = np.asarray(Wqr, dtype=np.float32)
    Wkr = np.asarray(Wkr, dtype=np.float32)
    Wkvd = np.asarray(Wkvd, dtype=np.float32)
    Wku = np.asarray(Wku, dtype=np.float32)
    Wvu = np.asarray(Wvu, dtype=np.float32)
    Wo = np.asarray(Wo, dtype=np.float32)
    lobo_log = np.asarray(lobo_log, dtype=np.float32)

    if "nc" not in _CACHE:
        _CACHE["nc"] = _build_program()
    nc = _CACHE["nc"]

    cosq, sinq = _rope_tables()
    msk = _masks()
    in_maps = []
    for core in range(8):
        b, g = core // NG, core % NG
        hs = slice(g * HL * DH, (g + 1) * HL * DH)
        # packed q-projection weights: per head [Wq_h*S | Wqr_h*S | h==0: Wkr]
        wq_pack = np.zeros((E, HL * 128), dtype=np.float32)
        for h in range(HL):
            gh = g * HL + h
            wq_pack[:, h * 128:h * 128 + DH] = Wq[:, gh * DH:(gh + 1) * DH] * SCALE
            wq_pack[:, h * 128 + DH:h * 128 + 96] = (
                Wqr[:, gh * DR:(gh + 1) * DR] * SCALE)
        wq_pack[:, 96:128] = Wkr
        cv = np.broadcast_to(
            np.exp(lobo_log[g * HL:(g + 1) * HL])[None, :], (32, HL))
        in_maps.append({
            "xT": _bf(x[b].T),
            "wq": _bf(wq_pack),
            "wkvd": _bf(Wkvd),
            "wku": _bf(Wku[:, hs]),
            "wvu": _bf(Wvu[:, hs]),
            "wo": _bf(Wo[hs, :]),
            "cosq": _bf(cosq), "sinq": _bf(sinq),
            "cvals": np.ascontiguousarray(cv, dtype=np.float32),
            "masks": _bf(msk),
            "ident": _bf(np.eye(128, dtype=np.float32)),
        })

    trace = bool(os.environ.get("BASS_TRACE_KERNEL"))
    bkr = run_bass_kernel_spmd(
        nc, in_maps, core_ids=list(range(8)), trace=trace)
    if trace:
        print(f"HW exec time: {bkr.exec_time_ns} ns")
        if bkr.instructions_and_trace is not None:
            print("trace:", bkr.instructions_and_trace[1])
        _CACHE["last_result"] = bkr
    res = bkr.results
    out = np.zeros((B, T, E), dtype=np.float32)
    for core in range(8):
        out[core // NG] += res[core]["out"].astype(np.float32)
    return out


# revision 18
# speedup vs baseline: 1.5950x; 1.0327x over previous
"""MLA (multi-head latent attention) Bass kernel for 8 trn2 NeuronCores — v2.

Sharding: core = b*4 + g  (b in {0,1} batches, g in {0..3} head-groups of 4 heads).

v2 redesign vs v1 (461us):
  - all inputs pre-cast to bf16 on HOST (halves load bytes; sync-ring HWDGE,
    no gpsimd cast DMAs), x loaded in per-(tg,ec) 128KB chunks so matmuls
    start early.
  - projections pack ONE head per 128-row PSUM slot: [q_c(64)|q_rope(32)|spare(32)]
    with Wkr riding head0's spare rows; rope half-swap via DVE stream_shuffle
    (32-lane permute) + TT ops reading PSUM directly -> zero SBUF<->SBUF
    staging DMAs.
  - attention: kc chunks processed in PAIRS sharing a [128,1024] 2-bank PSUM
    tile; ONE scalar exp per pair (scalar engine is the attention bottleneck).
    Causal masks split DVE/gpsimd. LOBO row-max tracked per (h,qg) in a comb
    tile; partition-reduce via DVE 32x32 transpose + shuffle-based 4-group
    max tree, all in a (j,b)-permuted [32,16] layout; denominator + reciprocal
    on [32,16] tiles; tiny perm<->natural DMAs on the idle sync ring.
  - normalize (bc broadcast matmul + DVE mul) software-pipelined one qg stage
    behind attention; output projection for head3's completed query blocks
    interleaves with head3's attention; out stored as bf16 partials
    (host all-reduces in f32).
"""

import math
import os

import numpy as np
import ml_dtypes

import concourse.bass as bass
import concourse.mybir as mybir
import concourse.tile as _tile_mod
from concourse.tile import TileContext
from concourse.vector_clock import ScopedClock, VectorClock
import bass_rust as _bass_rust
from concourse.bass_utils import run_bass_kernel_spmd

_N_PROCS = _bass_rust.N_PROCS
BF = ml_dtypes.bfloat16


def _split_drain_and_barrier(self, tick_clock, wait_clock):
    """Replacement for TileContext._drain_and_barrier: the stock version puts
    the whole global vector clock (up to 27 sem waits) on one Drain, which this
    walrus rejects ("Too many sync wait commands").  Emit one Drain per
    outstanding processor instead."""
    gc = tick_clock.global_clock
    procs = [p for p in range(_N_PROCS) if gc[p] > 0]
    for p in procs:
        vc = VectorClock([gc[q] if q == p else 0 for q in range(_N_PROCS)])
        d = self.nc.sync.drain()
        wait_clock.add_sem_waits(d.ins, ScopedClock({None: vc}))
    self.nc.all_engine_barrier()
    popped = self.nc._tile_sem_poison_stack.pop()
    assert popped is self._sem_poison
    self.nc.clear_and_free_semaphores(list(self.sems.allocated().values()))
    self.nc.all_engine_barrier()


_tile_mod.TileContext._drain_and_barrier = _split_drain_and_barrier

# ---------------------------------------------------------------------------
# This walrus build enforces small per-instruction sync-wait budgets
# ("Too many sync wait commands").  Post-process the BIR JSON: any
# instruction carrying more than its budget of waits gets the excess
# hoisted onto same-engine Drain carriers inserted immediately before it
# (same program point on the engine's sequential stream -> semantics
# unchanged).
# ---------------------------------------------------------------------------
_orig_to_json_bytes = bass.Bass.to_json_bytes
_WAIT_LIMITS = {"Drain": 1, "DMACopy": 1}
_DEF_WAIT_LIMIT = 1


def _to_json_split_waits(self, *a, **kw):
    import json as _json
    data = _json.loads(_orig_to_json_bytes(self, *a, **kw))
    nid = 0
    for f in data.get("functions", []):
        for bb in f.get("blocks", []):
            out = []
            for inst in bb.get("instructions", []):
                si = inst.get("sync_info")
                if isinstance(si, dict):
                    w = si.get("on_wait")
                    if isinstance(w, list):
                        k = _WAIT_LIMITS.get(inst.get("opcode"), _DEF_WAIT_LIMIT)
                        if len(w) > k:
                            extra, keep = w[:-k], w[-k:]
                            for wt in extra:
                                out.append({
                                    "debug": inst.get("debug"),
                                    "engine": inst["engine"],
                                    "ins": [], "outs": [],
                                    "name": f"wsplit-{nid}",
                                    "opcode": "NoOp",
                                    "sync_info": {"on_update": [],
                                                  "on_wait": [wt]},
                                })
                                nid += 1
                            si["on_wait"] = keep
                out.append(inst)
            bb["instructions"] = out
    return _json.dumps(data).encode()


bass.Bass.to_json_bytes = _to_json_split_waits

B, T, E = 2, 2048, 1024
H, DH = 16, 64
DKV = 256
DR = 32
HL = 4              # heads per core
NG = 4              # head groups
SCALE = 1.0 / math.sqrt(DH + DR)
TG = 512            # query-group width
KC = 128            # key-chunk width
NTG = T // TG       # 4
NKC = T // KC       # 16
EC = E // 128       # 8  e-chunks
CC = DKV // 128     # 2  latent chunks

F32 = mybir.dt.float32
BF16 = mybir.dt.bfloat16
AF = mybir.ActivationFunctionType
ALU = mybir.AluOpType
AX = mybir.AxisListType

_CACHE = {}

_ROT16 = [(i + 16) % 32 for i in range(32)]
_IDENT = list(range(32))


def _build_program():
    nc = bass.Bass()

    xT = nc.declare_dram_parameter("xT", [E, T], BF16, isOutput=False)
    wq = nc.declare_dram_parameter("wq", [E, HL * 128], BF16, isOutput=False)
    wkvd = nc.declare_dram_parameter("wkvd", [E, DKV], BF16, isOutput=False)
    wku = nc.declare_dram_parameter("wku", [DKV, HL * DH], BF16, isOutput=False)
    wvu = nc.declare_dram_parameter("wvu", [DKV, HL * DH], BF16, isOutput=False)
    wo = nc.declare_dram_parameter("wo", [HL * DH, E], BF16, isOutput=False)
    cosq = nc.declare_dram_parameter("cosq", [DR, T], BF16, isOutput=False)
    sinq = nc.declare_dram_parameter("sinq", [DR, T], BF16, isOutput=False)
    cvals = nc.declare_dram_parameter("cvals", [32, HL], F32, isOutput=False)
    masks = nc.declare_dram_parameter("masks", [128, 4 * TG], BF16, isOutput=False)
    ident = nc.declare_dram_parameter("ident", [128, 128], BF16, isOutput=False)
    out = nc.declare_dram_parameter("out", [T, E], BF16, isOutput=True)

    xTr = xT.rearrange("(c p) t -> p c t", p=128)

    with TileContext(nc) as tc:
        from contextlib import ExitStack

        with ExitStack() as ctx:
            singles = ctx.enter_context(tc.tile_pool(name="singles", bufs=1))
            pool = ctx.enter_context(tc.tile_pool(name="pool", bufs=2))
            psp = ctx.enter_context(tc.tile_pool(name="psp", bufs=1, space="PSUM"))

            # ---------------- weights + tables (all bf16 from host) --------
            wq_sb = singles.tile([128, EC, HL * 128], BF16)
            nc.sync.dma_start(out=wq_sb, in_=wq.rearrange("(c p) f -> p c f", p=128))
            wkvd_sb = singles.tile([128, EC, DKV], BF16)
            nc.sync.dma_start(
                out=wkvd_sb, in_=wkvd.rearrange("(c p) f -> p c f", p=128))
            cos_sb = singles.tile([DR, T], BF16)
            nc.sync.dma_start(out=cos_sb, in_=cosq[:, :])
            sin_sb = singles.tile([DR, T], BF16)
            nc.sync.dma_start(out=sin_sb, in_=sinq[:, :])
            wku_sb = singles.tile([128, CC, HL * DH], BF16)
            nc.sync.dma_start(out=wku_sb, in_=wku.rearrange("(c p) f -> p c f", p=128))
            wvu_sb = singles.tile([128, CC, HL * DH], BF16)
            nc.sync.dma_start(out=wvu_sb, in_=wvu.rearrange("(c p) f -> p c f", p=128))

            # x chunks for tg0 before the late-phase weights
            xt_sb = singles.tile([128, EC, T], BF16)
            for ec in range(EC):
                nc.sync.dma_start(out=xt_sb[:, ec, 0:TG], in_=xTr[:, ec, 0:TG])

            wo_sb = singles.tile([128, 2, E], BF16)
            nc.sync.dma_start(out=wo_sb, in_=wo.rearrange("(c p) e -> p c e", p=128))
            masks_sb = singles.tile([128, 4, TG], BF16)
            nc.sync.dma_start(
                out=masks_sb, in_=masks.rearrange("p (j y) -> p j y", j=4))
            ident_sb = singles.tile([128, 128], BF16)
            nc.sync.dma_start(out=ident_sb, in_=ident[:, :])
            onerow = singles.tile([1, TG], BF16)
            nc.vector.memset(onerow, 1.0)
            dext = singles.tile([32, TG], F32)
            nc.vector.memset(dext, 1.0)
            rstage = singles.tile([32, TG], BF16)
            nc.vector.memset(rstage, 1.0)
            c32_sb = singles.tile([32, HL], F32)
            nc.sync.dma_start(out=c32_sb, in_=cvals[:, :])

            for tg in range(1, NTG):
                ts = slice(tg * TG, (tg + 1) * TG)
                for ec in range(EC):
                    nc.sync.dma_start(out=xt_sb[:, ec, ts], in_=xTr[:, ec, ts])

            ones1 = singles.tile([1, DH], BF16)
            nc.vector.memset(ones1, 1.0)

            # ---------------- persistent activation tiles ----------------
            latT_sb = singles.tile([128, CC, T], BF16)
            qT = [singles.tile([96, T], BF16, name=f"qT{h}") for h in range(HL)]
            kT = [singles.tile([96, T], BF16, name=f"kT{h}") for h in range(HL)]
            v_sb = singles.tile([128, NKC, HL, DH + 1], BF16)
            nc.vector.memset(v_sb, 1.0)
            yraw_sb = singles.tile([DH, HL, T], BF16)
            yT_sb = singles.tile([128, 2, T], BF16)

            # ---------------- projections, per query-group ----------------
            for tg in range(NTG):
                ts = slice(tg * TG, (tg + 1) * TG)
                xts = [xt_sb[:, ec, ts] for ec in range(EC)]

                # q slots: two [128,1024] psum tiles, one head per 512-col half
                qps = []
                for hp in range(2):
                    sp = psp.tile([128, 2 * TG], F32, name=f"q{hp}", tag="S", bufs=2)
                    for hh in range(2):
                        h = 2 * hp + hh
                        fs = slice(h * 128, (h + 1) * 128)
                        for ec in range(EC):
                            nc.tensor.matmul(
                                sp[:, hh * TG:(hh + 1) * TG],
                                wq_sb[:, ec, fs], xts[ec],
                                start=(ec == 0), stop=(ec == EC - 1))
                    qps.append(sp)
                # latent: one [128,1024] psum tile (halves = latent rows 0:128/128:256)
                lp = psp.tile([128, 2 * TG], F32, name="lat", tag="S", bufs=2)
                for cc in range(CC):
                    for ec in range(EC):
                        nc.tensor.matmul(
                            lp[:, cc * TG:(cc + 1) * TG],
                            wkvd_sb[:, ec, cc * 128:(cc + 1) * 128], xts[ec],
                            start=(ec == 0), stop=(ec == EC - 1))

                # consumers: q_c copies (scalar), rope (DVE), latent (DVE)
                for h in range(HL):
                    sp = qps[h // 2]
                    hs = slice((h % 2) * TG, (h % 2 + 1) * TG)
                    nc.scalar.copy(qT[h][0:DH, ts], sp[0:DH, hs])

                def rope(dst, pre_ap):
                    sw = pool.tile([DR, TG], F32, name="sw", tag="qsw", bufs=2)
                    nc.vector.stream_shuffle(sw, pre_ap, _ROT16)
                    t1 = pool.tile([DR, TG], BF16, name="t1", tag="t1", bufs=2)
                    nc.vector.tensor_mul(t1, pre_ap, cos_sb[:, ts])
                    t2 = pool.tile([DR, TG], BF16, name="t2", tag="t2", bufs=2)
                    nc.vector.tensor_mul(t2, sw, sin_sb[:, ts])
                    nc.vector.tensor_add(dst, t1, t2)

                # head0 rope, then k_rope (both read qps[0] -> release it early)
                rope(qT[0][DH:96, ts], qps[0][DH:DH + DR, 0:TG])
                rope(kT[0][DH:96, ts], qps[0][96:128, 0:TG])
                for h in range(1, HL):
                    sp = qps[h // 2]
                    hs = slice((h % 2) * TG, (h % 2 + 1) * TG)
                    rope(qT[h][DH:96, ts], sp[DH:DH + DR, hs])
                for h in range(1, HL):
                    nc.scalar.copy(kT[h][DH:96, ts], kT[0][DH:96, ts])
                # latent -> SBUF bf16
                nc.vector.tensor_copy(latT_sb[:, 0, ts], lp[:, 0:TG])
                nc.vector.tensor_copy(latT_sb[:, 1, ts], lp[:, TG:2 * TG])

                # k_c = Wku^T latent (per head-pair), consumed by scalar copies
                for hp in range(2):
                    mp = psp.tile([128, TG], F32, name="kc", tag="M", bufs=2)
                    for cc in range(CC):
                        nc.tensor.matmul(
                            mp, wku_sb[:, cc, hp * 128:(hp + 1) * 128],
                            latT_sb[:, cc, ts],
                            start=(cc == 0), stop=(cc == CC - 1))
                    nc.scalar.copy(kT[2 * hp][0:DH, ts], mp[0:DH, :])
                    nc.scalar.copy(kT[2 * hp + 1][0:DH, ts], mp[DH:128, :])

                # V natural layout
                for tt in range(4 * tg, 4 * tg + 4):
                    bp = psp.tile([128, HL * DH], F32, name="v", tag="B", bufs=2)
                    for cc in range(CC):
                        nc.tensor.matmul(
                            bp, latT_sb[:, cc, tt * 128:(tt + 1) * 128],
                            wvu_sb[:, cc, :],
                            start=(cc == 0), stop=(cc == CC - 1))
                    nc.vector.tensor_copy(v_sb[:, tt, :, 0:DH], bp)

            # ---------------- attention ----------------
            # bc+yn for stage (h,qg) is emitted one iteration later so the
            # epilogue's DVE chain never stalls the tensor queue; head3's
            # finished query blocks trigger the output projection inline.
            norm_queue = []

            def emit_norm(h, qg, rback):
                qs = slice(qg * TG, (qg + 1) * TG)
                bcp = psp.tile([DH, TG], F32, name="bc", tag="B", bufs=2)
                nc.tensor.matmul(bcp, ones1, rback[0:1, :])
                nc.vector.tensor_mul(
                    yT_sb[(h % 2) * DH:(h % 2 + 1) * DH, h // 2, qs],
                    yraw_sb[0:DH, h, qs], bcp)

            def emit_outproj(qg):
                for tt in range(4 * qg, 4 * qg + 4):
                    op = psp.tile([128, E], F32, name="op", tag="S", bufs=2)
                    for eg in range(2):
                        for fc in range(2):
                            nc.tensor.matmul(
                                op[:, eg * TG:(eg + 1) * TG],
                                yT_sb[:, fc, tt * 128:(tt + 1) * 128],
                                wo_sb[:, fc, eg * TG:(eg + 1) * TG],
                                start=(fc == 0), stop=(fc == 1))
                    ost = pool.tile([128, E], BF16, name="ost", tag="ost", bufs=2)
                    nc.vector.tensor_copy(ost, op)
                    nc.sync.dma_start(
                        out=out[tt * 128:(tt + 1) * 128, :], in_=ost)

            for h in range(HL):
                for qg in range(NTG):
                    qs = slice(qg * TG, (qg + 1) * TG)
                    nkc = 4 * qg + 4
                    npair = nkc // 2
                    yps = psp.tile([DH + 1, TG], F32, name="py", tag="M", bufs=2)
                    comb = pool.tile([128, TG], BF16, name="comb", tag="comb",
                                     bufs=2)
                    for pr in range(npair):
                        kca, kcb = 2 * pr, 2 * pr + 1
                        diag = 4 * qg  # first diagonal chunk
                        sp = psp.tile([128, 2 * TG], F32, name="ps", tag="S",
                                      bufs=2)
                        for hv, kc in ((0, kca), (1, kcb)):
                            half = sp[:, hv * TG:(hv + 1) * TG]
                            j = kc - diag
                            nc.tensor.matmul(
                                half, kT[h][:, kc * KC:(kc + 1) * KC],
                                qT[h][:, qs],
                                start=True, stop=(j < 0))
                            if j >= 0:
                                # additive causal mask: += ident^T @ (-1e4 pattern)
                                w = 128 * j + 128
                                nc.tensor.matmul(
                                    sp[:, hv * TG:hv * TG + w],
                                    ident_sb, masks_sb[:, j, 0:w],
                                    start=False, stop=True)
                        pt = pool.tile([128, 2 * TG], BF16, name="pt", tag="pt",
                                       bufs=3)
                        nc.scalar.activation(pt, sp, AF.Exp)
                        if pr == 0:
                            nc.vector.tensor_max(comb, pt[:, 0:TG], pt[:, TG:2 * TG])
                        else:
                            nc.vector.tensor_max(comb, comb, pt[:, 0:TG])
                            nc.vector.tensor_max(comb, comb, pt[:, TG:2 * TG])
                        nc.tensor.matmul(
                            yps, v_sb[:, kca, h, :], pt[:, 0:TG],
                            start=(pr == 0), stop=False)
                        nc.tensor.matmul(
                            yps, v_sb[:, kcb, h, :], pt[:, TG:2 * TG],
                            start=False, stop=(pr == npair - 1))
                    # epilogue for (h, qg): emax via transpose+shuffle tree (perm
                    # (j,b) layout), d + reciprocal via transpose round-trip.
                    nc.vector.tensor_copy(yraw_sb[:, h, qs], yps[0:DH, :])
                    nc.vector.tensor_mul(dext[0:1, :], yps[DH:DH + 1, :], onerow)
                    combT = pool.tile([128, TG], BF16, name="combT", tag="combT",
                                      bufs=2)
                    nc.vector.transpose(combT, comb)
                    red = pool.tile([128, 16], F32, name="red", tag="red", bufs=2)
                    nc.vector.reduce_max(
                        red, combT.rearrange("p (b j) -> p b j", j=32), axis=AX.X)
                    s1 = pool.tile([32, 16], F32, name="s1", tag="s1", bufs=2)
                    nc.vector.stream_shuffle(s1, red[32:64, :], _IDENT)
                    s2 = pool.tile([32, 16], F32, name="s2", tag="s2", bufs=2)
                    nc.vector.stream_shuffle(s2, red[64:96, :], _IDENT)
                    s3 = pool.tile([32, 16], F32, name="s3", tag="s3", bufs=2)
                    nc.vector.stream_shuffle(s3, red[96:128, :], _IDENT)
                    nc.vector.tensor_max(s1, red[0:32, :], s1)
                    nc.vector.tensor_max(s2, s2, s3)
                    emfP = pool.tile([32, 16], F32, name="emf", tag="emf", bufs=2)
                    nc.vector.tensor_max(emfP, s1, s2)
                    dTt = pool.tile([32, TG], F32, name="dTt", tag="dTt", bufs=2)
                    nc.vector.transpose(dTt, dext)
                    dP = pool.tile([32, 16], F32, name="dP", tag="dP", bufs=2)
                    nc.vector.scalar_tensor_tensor(
                        out=dP, in0=emfP, scalar=c32_sb[:, h:h + 1],
                        in1=dTt.rearrange("p (b j) -> p b j", j=32)[:, :, 0],
                        op0=ALU.mult, op1=ALU.add)
                    with nc.allow_low_precision(reason="r in bf16; tol 2e-2"):
                        nc.vector.reciprocal(
                            rstage.rearrange("p (b j) -> p b j", j=32)[:, :, 0], dP)
                    rback = pool.tile([32, TG], BF16, name="rbk", tag="rbk", bufs=2)
                    nc.vector.transpose(rback, rstage)
                    # queue normalization; drain one stage behind
                    norm_queue.append((h, qg, rback))
                    if len(norm_queue) > 1:
                        hq, qq, r_ = norm_queue.pop(0)
                        emit_norm(hq, qq, r_)
                        if hq == 3:
                            emit_outproj(qq)
            while norm_queue:
                hq, qq, r_ = norm_queue.pop(0)
                emit_norm(hq, qq, r_)
                if hq == 3:
                    emit_outproj(qq)

    return nc


def _masks():
    """Additive causal masks: -1e4 on masked (k > q) cells, 0 elsewhere."""
    x = np.arange(128)[:, None]
    y = np.arange(TG)[None, :]
    ms = [np.where(x - y + 128 * j <= 0, 0.0, -1e4).astype(np.float32)
          for j in range(4)]
    return np.concatenate(ms, axis=1)  # [128, 4*TG]


def _rope_tables():
    half = DR // 2
    inv = 1.0 / (10000.0 ** (np.arange(half, dtype=np.float64) / half))
    ang = np.arange(T, dtype=np.float64)[:, None] * inv[None, :]  # (T, half)
    cos = np.cos(ang).T  # (half, T)
    sin = np.sin(ang).T
    cosk = np.concatenate([cos, cos], axis=0)                 # (32, T)
    sink = np.concatenate([-sin, sin], axis=0)
    return cosk.astype(np.float32), sink.astype(np.float32)


def _bf(x):
    return np.ascontiguousarray(np.asarray(x, dtype=np.float32).astype(BF))


def kernel(x, Wq, Wqr, Wkr, Wkvd, Wku, Wvu, Wo, lobo_log):
    x = np.asarray(x, dtype=np.float32)
    Wq = np.asarray(Wq, dtype=np.float32)
    Wqr = np.asarray(Wqr, dtype=np.float32)
    Wkr = np.asarray(Wkr, dtype=np.float32)
    Wkvd = np.asarray(Wkvd, dtype=np.float32)
    Wku = np.asarray(Wku, dtype=np.float32)
    Wvu = np.asarray(Wvu, dtype=np.float32)
    Wo = np.asarray(Wo, dtype=np.float32)
    lobo_log = np.asarray(lobo_log, dtype=np.float32)

    if "nc" not in _CACHE:
        _CACHE["nc"] = _build_program()
    nc = _CACHE["nc"]

    cosq, sinq = _rope_tables()
    msk = _masks()
    in_maps = []
    for core in range(8):
        b, g = core // NG, core % NG
        hs = slice(g * HL * DH, (g + 1) * HL * DH)
        # packed q-projection weights: per head [Wq_h*S | Wqr_h*S | h==0: Wkr]
        wq_pack = np.zeros((E, HL * 128), dtype=np.float32)
        for h in range(HL):
            gh = g * HL + h
            wq_pack[:, h * 128:h * 128 + DH] = Wq[:, gh * DH:(gh + 1) * DH] * SCALE
            wq_pack[:, h * 128 + DH:h * 128 + 96] = (
                Wqr[:, gh * DR:(gh + 1) * DR] * SCALE)
        wq_pack[:, 96:128] = Wkr
        cv = np.broadcast_to(
            np.exp(lobo_log[g * HL:(g + 1) * HL])[None, :], (32, HL))
        in_maps.append({
            "xT": _bf(x[b].T),
            "wq": _bf(wq_pack),
            "wkvd": _bf(Wkvd),
            "wku": _bf(Wku[:, hs]),
            "wvu": _bf(Wvu[:, hs]),
            "wo": _bf(Wo[hs, :]),
            "cosq": _bf(cosq), "sinq": _bf(sinq),
            "cvals": np.ascontiguousarray(cv, dtype=np.float32),
            "masks": _bf(msk),
            "ident": _bf(np.eye(128, dtype=np.float32)),
        })

    trace = bool(os.environ.get("BASS_TRACE_KERNEL"))
    bkr = run_bass_kernel_spmd(
        nc, in_maps, core_ids=list(range(8)), trace=trace)
    if trace:
        print(f"HW exec time: {bkr.exec_time_ns} ns")
        if bkr.instructions_and_trace is not None:
            print("trace:", bkr.instructions_and_trace[1])
        _CACHE["last_result"] = bkr
    res = bkr.results
    out = np.zeros((B, T, E), dtype=np.float32)
    for core in range(8):
        out[core // NG] += res[core]["out"].astype(np.float32)
    return out


# revision 25
# speedup vs baseline: 1.7911x; 1.1229x over previous
"""MLA (multi-head latent attention) Bass kernel for 8 trn2 NeuronCores — v2.

Sharding: core = b*4 + g  (b in {0,1} batches, g in {0..3} head-groups of 4 heads).

v2 redesign vs v1 (461us):
  - all inputs pre-cast to bf16 on HOST (halves load bytes; sync-ring HWDGE,
    no gpsimd cast DMAs), x loaded in per-(tg,ec) 128KB chunks so matmuls
    start early.
  - projections pack ONE head per 128-row PSUM slot: [q_c(64)|q_rope(32)|spare(32)]
    with Wkr riding head0's spare rows; rope half-swap via DVE stream_shuffle
    (32-lane permute) + TT ops reading PSUM directly -> zero SBUF<->SBUF
    staging DMAs.
  - attention: kc chunks processed in PAIRS sharing a [128,1024] 2-bank PSUM
    tile; ONE scalar exp per pair (scalar engine is the attention bottleneck).
    Causal masks split DVE/gpsimd. LOBO row-max tracked per (h,qg) in a comb
    tile; partition-reduce via DVE 32x32 transpose + shuffle-based 4-group
    max tree, all in a (j,b)-permuted [32,16] layout; denominator + reciprocal
    on [32,16] tiles; tiny perm<->natural DMAs on the idle sync ring.
  - normalize (bc broadcast matmul + DVE mul) software-pipelined one qg stage
    behind attention; output projection for head3's completed query blocks
    interleaves with head3's attention; out stored as bf16 partials
    (host all-reduces in f32).
"""

import math
import os

import numpy as np
import ml_dtypes

import concourse.bass as bass
import concourse.mybir as mybir
import concourse.tile as _tile_mod
from concourse.tile import TileContext
from concourse.vector_clock import ScopedClock, VectorClock
import bass_rust as _bass_rust
from concourse.bass_utils import run_bass_kernel_spmd

_N_PROCS = _bass_rust.N_PROCS
BF = ml_dtypes.bfloat16


def _split_drain_and_barrier(self, tick_clock, wait_clock):
    """Replacement for TileContext._drain_and_barrier: the stock version puts
    the whole global vector clock (up to 27 sem waits) on one Drain, which this
    walrus rejects ("Too many sync wait commands").  Emit one Drain per
    outstanding processor instead."""
    gc = tick_clock.global_clock
    procs = [p for p in range(_N_PROCS) if gc[p] > 0]
    for p in procs:
        vc = VectorClock([gc[q] if q == p else 0 for q in range(_N_PROCS)])
        d = self.nc.sync.drain()
        wait_clock.add_sem_waits(d.ins, ScopedClock({None: vc}))
    self.nc.all_engine_barrier()
    popped = self.nc._tile_sem_poison_stack.pop()
    assert popped is self._sem_poison
    self.nc.clear_and_free_semaphores(list(self.sems.allocated().values()))
    self.nc.all_engine_barrier()


_tile_mod.TileContext._drain_and_barrier = _split_drain_and_barrier

# ---------------------------------------------------------------------------
# This walrus build enforces small per-instruction sync-wait budgets
# ("Too many sync wait commands").  Post-process the BIR JSON: any
# instruction carrying more than its budget of waits gets the excess
# hoisted onto same-engine Drain carriers inserted immediately before it
# (same program point on the engine's sequential stream -> semantics
# unchanged).
# ---------------------------------------------------------------------------
_orig_to_json_bytes = bass.Bass.to_json_bytes
_WAIT_LIMITS = {"Drain": 1, "DMACopy": 1}
_DEF_WAIT_LIMIT = 1


def _to_json_split_waits(self, *a, **kw):
    import json as _json
    data = _json.loads(_orig_to_json_bytes(self, *a, **kw))
    nid = 0
    for f in data.get("functions", []):
        for bb in f.get("blocks", []):
            out = []
            for inst in bb.get("instructions", []):
                si = inst.get("sync_info")
                if isinstance(si, dict):
                    w = si.get("on_wait")
                    if isinstance(w, list):
                        k = _WAIT_LIMITS.get(inst.get("opcode"), _DEF_WAIT_LIMIT)
                        if len(w) > k:
                            extra, keep = w[:-k], w[-k:]
                            for wt in extra:
                                out.append({
                                    "debug": inst.get("debug"),
                                    "engine": inst["engine"],
                                    "ins": [], "outs": [],
                                    "name": f"wsplit-{nid}",
                                    "opcode": "NoOp",
                                    "sync_info": {"on_update": [],
                                                  "on_wait": [wt]},
                                })
                                nid += 1
                            si["on_wait"] = keep
                out.append(inst)
            bb["instructions"] = out
    return _json.dumps(data).encode()


bass.Bass.to_json_bytes = _to_json_split_waits

B, T, E = 2, 2048, 1024
H, DH = 16, 64
DKV = 256
DR = 32
HL = 4              # heads per core
NG = 4              # head groups
SCALE = 1.0 / math.sqrt(DH + DR)
TG = 512            # query-group width
KC = 128            # key-chunk width
NTG = T // TG       # 4
NKC = T // KC       # 16
EC = E // 128       # 8  e-chunks
CC = DKV // 128     # 2  latent chunks

F32 = mybir.dt.float32
BF16 = mybir.dt.bfloat16
AF = mybir.ActivationFunctionType
ALU = mybir.AluOpType
AX = mybir.AxisListType

_CACHE = {}

_ROT16 = [(i + 16) % 32 for i in range(32)]
_IDENT = list(range(32))


def _build_program():
    nc = bass.Bass()

    xT = nc.declare_dram_parameter("xT", [E, T], BF16, isOutput=False)
    wq = nc.declare_dram_parameter("wq", [E, HL * 128], BF16, isOutput=False)
    wkvd = nc.declare_dram_parameter("wkvd", [E, DKV], BF16, isOutput=False)
    wku = nc.declare_dram_parameter("wku", [DKV, HL * DH], BF16, isOutput=False)
    wvu = nc.declare_dram_parameter("wvu", [DKV, HL * DH], BF16, isOutput=False)
    wo = nc.declare_dram_parameter("wo", [HL * DH, E], BF16, isOutput=False)
    cosq = nc.declare_dram_parameter("cosq", [128, T], BF16, isOutput=False)
    sinq = nc.declare_dram_parameter("sinq", [128, T], BF16, isOutput=False)
    cvals = nc.declare_dram_parameter("cvals", [32, HL], F32, isOutput=False)
    masks = nc.declare_dram_parameter("masks", [128, 4 * TG], BF16, isOutput=False)
    ident = nc.declare_dram_parameter("ident", [128, 128], BF16, isOutput=False)
    out = nc.declare_dram_parameter("out", [T, E], BF16, isOutput=True)

    xTr = xT.rearrange("(c p) t -> p c t", p=128)

    with TileContext(nc) as tc:
        from contextlib import ExitStack

        with ExitStack() as ctx:
            singles = ctx.enter_context(tc.tile_pool(name="singles", bufs=1))
            pool = ctx.enter_context(tc.tile_pool(name="pool", bufs=2))
            psp = ctx.enter_context(tc.tile_pool(name="psp", bufs=1, space="PSUM"))

            # ---------------- weights + tables (all bf16 from host) --------
            wq_sb = singles.tile([128, EC, HL * 128], BF16)
            nc.sync.dma_start(out=wq_sb, in_=wq.rearrange("(c p) f -> p c f", p=128))
            wkvd_sb = singles.tile([128, EC, DKV], BF16)
            nc.sync.dma_start(
                out=wkvd_sb, in_=wkvd.rearrange("(c p) f -> p c f", p=128))
            cos_sb = singles.tile([128, T], BF16)
            nc.sync.dma_start(out=cos_sb, in_=cosq[:, :])
            sin_sb = singles.tile([128, T], BF16)
            nc.sync.dma_start(out=sin_sb, in_=sinq[:, :])
            wku_sb = singles.tile([128, CC, HL * DH], BF16)
            nc.sync.dma_start(out=wku_sb, in_=wku.rearrange("(c p) f -> p c f", p=128))
            wvu_sb = singles.tile([128, CC, HL * DH], BF16)
            nc.sync.dma_start(out=wvu_sb, in_=wvu.rearrange("(c p) f -> p c f", p=128))

            # x chunks: pool ring, 2 query-groups in flight
            xt_tiles = {}

            def load_xt(tg):
                ts_ = slice(tg * TG, (tg + 1) * TG)
                for ec_ in range(EC):
                    t_ = pool.tile([128, TG], BF16, name=f"xt{tg}_{ec_}",
                                   tag="xt", bufs=16)
                    nc.sync.dma_start(out=t_, in_=xTr[:, ec_, ts_])
                    xt_tiles[(tg, ec_)] = t_

            load_xt(0)

            wo_sb = singles.tile([128, 2, E], BF16)
            nc.sync.dma_start(out=wo_sb, in_=wo.rearrange("(c p) e -> p c e", p=128))
            masks_sb = singles.tile([128, 4, TG], BF16)
            nc.sync.dma_start(
                out=masks_sb, in_=masks.rearrange("p (j y) -> p j y", j=4))
            ident_sb = singles.tile([128, 128], BF16)
            nc.sync.dma_start(out=ident_sb, in_=ident[:, :])
            onerow = singles.tile([1, TG], BF16)
            nc.vector.memset(onerow, 1.0)
            dext = singles.tile([32, T], F32)
            nc.vector.memset(dext, 1.0)
            rstage = singles.tile([32, T], BF16)
            nc.vector.memset(rstage, 1.0)
            c32_sb = singles.tile([32, HL], F32)
            nc.sync.dma_start(out=c32_sb, in_=cvals[:, :])

            for tg in range(1, NTG):
                load_xt(tg)

            ones1 = singles.tile([1, DH], BF16)
            nc.vector.memset(ones1, 1.0)

            # ---------------- persistent activation tiles ----------------
            latT_sb = singles.tile([128, CC, T], BF16)
            qT = [singles.tile([96, T], BF16, name=f"qT{h}") for h in range(HL)]
            kT = [singles.tile([96, T], BF16, name=f"kT{h}") for h in range(HL)]
            v_sb = singles.tile([128, NKC, HL, DH + 1], BF16)
            nc.vector.memset(v_sb, 1.0)
            yraw_sb = singles.tile([DH, HL, T], BF16)
            yT_sb = singles.tile([128, 2, T], BF16)

            # ---------------- projections, per query-group ----------------
            # weight column blocks: A=[qc_h0|qc_h1] B=[qc_h2|qc_h3]
            #                       C=[qr_h0..h3 (4x32)] D=[kr(32)|zeros]
            for tg in range(NTG):
                ts = slice(tg * TG, (tg + 1) * TG)
                xts = [xt_tiles[(tg, ec)] for ec in range(EC)]

                def proj_pair(blk0, blk1, name):
                    sp = psp.tile([128, 2 * TG], F32, name=name, tag="S", bufs=2)
                    for hv, blk in ((0, blk0), (1, blk1)):
                        fs = slice(blk * 128, (blk + 1) * 128)
                        for ec in range(EC):
                            nc.tensor.matmul(
                                sp[:, hv * TG:(hv + 1) * TG],
                                wq_sb[:, ec, fs], xts[ec],
                                start=(ec == 0), stop=(ec == EC - 1))
                    return sp

                qcp = proj_pair(0, 1, "qc")    # qc h0|h1 , h2|h3
                qrp = proj_pair(2, 3, "qr")    # qr all-heads , kr
                lp = psp.tile([128, 2 * TG], F32, name="lat", tag="S", bufs=2)
                for cc in range(CC):
                    for ec in range(EC):
                        nc.tensor.matmul(
                            lp[:, cc * TG:(cc + 1) * TG],
                            wkvd_sb[:, ec, cc * 128:(cc + 1) * 128], xts[ec],
                            start=(ec == 0), stop=(ec == EC - 1))

                # q_c: even heads on DVE (equal-base PSUM copy), odd on scalar
                nc.vector.tensor_copy(qT[0][0:DH, ts], qcp[0:DH, 0:TG])
                nc.scalar.copy(qT[1][0:DH, ts], qcp[DH:128, 0:TG])
                nc.vector.tensor_copy(qT[2][0:DH, ts], qcp[0:DH, TG:2 * TG])
                nc.scalar.copy(qT[3][0:DH, ts], qcp[DH:128, TG:2 * TG])

                # q rope, all 4 heads in one [128,512] batch
                sw = pool.tile([128, TG], F32, name="sw", tag="qsw", bufs=2)
                nc.vector.stream_shuffle(sw, qrp[:, 0:TG], _ROT16)
                t1 = pool.tile([128, TG], BF16, name="t1", tag="t1", bufs=2)
                nc.vector.tensor_mul(t1, qrp[:, 0:TG], cos_sb[:, ts])
                t2 = pool.tile([128, TG], BF16, name="t2", tag="t2", bufs=2)
                nc.vector.tensor_mul(t2, sw, sin_sb[:, ts])
                rq = pool.tile([128, TG], BF16, name="rq", tag="rq", bufs=2)
                nc.vector.tensor_add(rq, t1, t2)
                for h in range(HL):
                    nc.scalar.copy(qT[h][DH:96, ts], rq[h * DR:(h + 1) * DR, :])
                # k rope (rows 0:32 of the D half)
                sw2 = pool.tile([DR, TG], F32, name="sw2", tag="sw2", bufs=2)
                nc.vector.stream_shuffle(sw2, qrp[0:DR, TG:2 * TG], _ROT16)
                k1 = pool.tile([DR, TG], BF16, name="k1", tag="k1", bufs=2)
                nc.vector.tensor_mul(k1, qrp[0:DR, TG:2 * TG], cos_sb[0:DR, ts])
                k2 = pool.tile([DR, TG], BF16, name="k2", tag="k2", bufs=2)
                nc.vector.tensor_mul(k2, sw2, sin_sb[0:DR, ts])
                nc.vector.tensor_add(kT[0][DH:96, ts], k1, k2)
                for h in range(1, HL):
                    nc.scalar.copy(kT[h][DH:96, ts], kT[0][DH:96, ts])
                # latent -> SBUF bf16
                nc.vector.tensor_copy(latT_sb[:, 0, ts], lp[:, 0:TG])
                nc.vector.tensor_copy(latT_sb[:, 1, ts], lp[:, TG:2 * TG])

                # k_c = Wku^T latent (per head-pair)
                for hp in range(2):
                    mp = psp.tile([128, TG], F32, name="kc", tag="M", bufs=2)
                    for cc in range(CC):
                        nc.tensor.matmul(
                            mp, wku_sb[:, cc, hp * 128:(hp + 1) * 128],
                            latT_sb[:, cc, ts],
                            start=(cc == 0), stop=(cc == CC - 1))
                    nc.vector.tensor_copy(kT[2 * hp][0:DH, ts], mp[0:DH, :])
                    nc.scalar.copy(kT[2 * hp + 1][0:DH, ts], mp[DH:128, :])

                # V natural layout
                for tt in range(4 * tg, 4 * tg + 4):
                    bp = psp.tile([128, HL * DH], F32, name="v", tag="B", bufs=2)
                    for cc in range(CC):
                        nc.tensor.matmul(
                            bp, latT_sb[:, cc, tt * 128:(tt + 1) * 128],
                            wvu_sb[:, cc, :],
                            start=(cc == 0), stop=(cc == CC - 1))
                    nc.vector.tensor_copy(v_sb[:, tt, :, 0:DH], bp)

            # ---------------- attention ----------------
            # bc+yn for stage (h,qg) is emitted one iteration later so the
            # epilogue's DVE chain never stalls the tensor queue; head3's
            # finished query blocks trigger the output projection inline.
            norm_queue = []

            def emit_norm(h, qg, rback):
                qs = slice(qg * TG, (qg + 1) * TG)
                bcp = psp.tile([DH, TG], F32, name="bc", tag="B", bufs=2)
                nc.tensor.matmul(bcp, ones1, rback[0:1, :])
                nc.vector.tensor_mul(
                    yT_sb[(h % 2) * DH:(h % 2 + 1) * DH, h // 2, qs],
                    yraw_sb[0:DH, h, qs], bcp)

            def emit_outproj(qg):
                for tt in range(4 * qg, 4 * qg + 4):
                    op = psp.tile([128, E], F32, name="op", tag="S", bufs=2)
                    for eg in range(2):
                        for fc in range(2):
                            nc.tensor.matmul(
                                op[:, eg * TG:(eg + 1) * TG],
                                yT_sb[:, fc, tt * 128:(tt + 1) * 128],
                                wo_sb[:, fc, eg * TG:(eg + 1) * TG],
                                start=(fc == 0), stop=(fc == 1))
                    ost = pool.tile([128, E], BF16, name="ost", tag="ost", bufs=2)
                    nc.vector.tensor_copy(ost, op)
                    nc.sync.dma_start(
                        out=out[tt * 128:(tt + 1) * 128, :], in_=ost)

            def emax_dr(h, cols, comb_ap, dext_ap, rstage_ap):
                """emax (transpose + reduce + shuffle-tree, (j,b)-perm layout),
                then d = dsum + C*emax and 1/d via a transpose round-trip.
                cols = number of columns (512 per-qg or 2048 per-head)."""
                nb = cols // 32
                combT = pool.tile([128, cols], BF16, name="combT",
                                  tag=f"combT{cols}", bufs=1)
                nc.vector.transpose(combT, comb_ap)
                red = pool.tile([128, nb], F32, name="red", tag=f"red{cols}",
                                bufs=2)
                nc.vector.reduce_max(
                    red, combT.rearrange("p (b j) -> p b j", j=32), axis=AX.X)
                s1 = pool.tile([32, nb], F32, name="s1", tag=f"s1{cols}", bufs=2)
                nc.vector.stream_shuffle(s1, red[32:64, :], _IDENT)
                s2 = pool.tile([32, nb], F32, name="s2", tag=f"s2{cols}", bufs=2)
                nc.vector.stream_shuffle(s2, red[64:96, :], _IDENT)
                s3 = pool.tile([32, nb], F32, name="s3", tag=f"s3{cols}", bufs=2)
                nc.vector.stream_shuffle(s3, red[96:128, :], _IDENT)
                nc.vector.tensor_max(s1, red[0:32, :], s1)
                nc.vector.tensor_max(s2, s2, s3)
                emfP = pool.tile([32, nb], F32, name="emf", tag=f"emf{cols}",
                                 bufs=2)
                nc.vector.tensor_max(emfP, s1, s2)
                dTt = pool.tile([32, cols], F32, name="dTt", tag=f"dTt{cols}",
                                bufs=1)
                nc.vector.transpose(dTt, dext_ap)
                dP = pool.tile([32, nb], F32, name="dP", tag=f"dP{cols}", bufs=2)
                nc.vector.scalar_tensor_tensor(
                    out=dP, in0=emfP, scalar=c32_sb[:, h:h + 1],
                    in1=dTt.rearrange("p (b j) -> p b j", j=32)[:, :, 0],
                    op0=ALU.mult, op1=ALU.add)
                with nc.allow_low_precision(reason="r in bf16; tol 2e-2"):
                    nc.vector.reciprocal(
                        rstage_ap.rearrange("p (b j) -> p b j", j=32)[:, :, 0], dP)
                rback = pool.tile([32, cols], BF16, name="rbk",
                                  tag=f"rbk{cols}", bufs=2)
                nc.vector.transpose(rback, rstage_ap)
                return rback

            def drain_norms(n):
                for _ in range(n):
                    if norm_queue:
                        norm_queue.pop(0)()

            for h in range(HL):
                comb = pool.tile([128, T], BF16, name="comb", tag="comb", bufs=2)
                for qg in range(NTG):
                    qs = slice(qg * TG, (qg + 1) * TG)
                    qsB = slice((qg - 2) * TG, (qg - 1) * TG)
                    nkc = 4 * qg + 4
                    npair = nkc // 2
                    yps = psp.tile([DH + 1, TG], F32, name="py", tag="M", bufs=2)
                    dve_first = gp_first = True
                    for pr in range(npair):
                        kca, kcb = 2 * pr, 2 * pr + 1
                        diag = 4 * qg  # first diagonal chunk
                        sp = psp.tile([128, 2 * TG], F32, name="ps", tag="S",
                                      bufs=2)
                        for hv, kc in ((0, kca), (1, kcb)):
                            half = sp[:, hv * TG:(hv + 1) * TG]
                            j = kc - diag
                            nc.tensor.matmul(
                                half, kT[h][:, kc * KC:(kc + 1) * KC],
                                qT[h][:, qs],
                                start=True, stop=(j < 0))
                            if j >= 0:
                                # additive causal mask: += ident^T @ (-1e4 pattern)
                                w = 128 * j + 128
                                nc.tensor.matmul(
                                    sp[:, hv * TG:hv * TG + w],
                                    ident_sb, masks_sb[:, j, 0:w],
                                    start=False, stop=True)
                        pt = pool.tile([128, 2 * TG], BF16, name="pt", tag="pt",
                                       bufs=3)
                        nc.scalar.activation(pt, sp, AF.Exp)
                        # row-max tracking (gpsimd lacks a max op: DVE only)
                        if dve_first:
                            nc.vector.tensor_max(
                                comb[:, qs], pt[:, 0:TG], pt[:, TG:2 * TG])
                            dve_first = False
                        else:
                            nc.vector.tensor_max(
                                comb[:, qs], comb[:, qs], pt[:, 0:TG])
                            nc.vector.tensor_max(
                                comb[:, qs], comb[:, qs], pt[:, TG:2 * TG])
                        nc.tensor.matmul(
                            yps, v_sb[:, kca, h, :], pt[:, 0:TG],
                            start=(pr == 0), stop=False)
                        nc.tensor.matmul(
                            yps, v_sb[:, kcb, h, :], pt[:, TG:2 * TG],
                            start=False, stop=(pr == npair - 1))
                    # stage epilogue
                    nc.vector.tensor_copy(yraw_sb[:, h, qs], yps[0:DH, :])
                    nc.scalar.copy(dext[0:1, qs], yps[DH:DH + 1, :])
                    if h == 3:
                        rback = emax_dr(h, TG, comb[:, qs], dext[:, qs],
                                        rstage[:, qs])
                        norm_queue.append(
                            (lambda hh=h, qq=qg, r_=rback:
                             (emit_norm(hh, qq, r_), emit_outproj(qq))))
                    drain_norms(2 if h == 3 else 1)
                if h < 3:
                    rback = emax_dr(h, T, comb, dext, rstage)
                    for qg in range(NTG):
                        norm_queue.append(
                            (lambda hh=h, qq=qg,
                             r_=rback[:, qg * TG:(qg + 1) * TG]:
                             emit_norm(hh, qq, r_)))
            drain_norms(len(norm_queue))

    return nc


def _masks():
    """Additive causal masks: -1e4 on masked (k > q) cells, 0 elsewhere."""
    x = np.arange(128)[:, None]
    y = np.arange(TG)[None, :]
    ms = [np.where(x - y + 128 * j <= 0, 0.0, -1e4).astype(np.float32)
          for j in range(4)]
    return np.concatenate(ms, axis=1)  # [128, 4*TG]


def _rope_tables():
    half = DR // 2
    inv = 1.0 / (10000.0 ** (np.arange(half, dtype=np.float64) / half))
    ang = np.arange(T, dtype=np.float64)[:, None] * inv[None, :]  # (T, half)
    cos = np.cos(ang).T  # (half, T)
    sin = np.sin(ang).T
    cosk = np.concatenate([cos, cos], axis=0)                 # (32, T)
    sink = np.concatenate([-sin, sin], axis=0)
    cosq = np.tile(cosk, (HL, 1))                             # (128, T)
    sinq = np.tile(sink, (HL, 1))
    return cosq.astype(np.float32), sinq.astype(np.float32)


def _bf(x):
    return np.ascontiguousarray(np.asarray(x, dtype=np.float32).astype(BF))


def kernel(x, Wq, Wqr, Wkr, Wkvd, Wku, Wvu, Wo, lobo_log):
    x = np.asarray(x, dtype=np.float32)
    Wq = np.asarray(Wq, dtype=np.float32)
    Wqr = np.asarray(Wqr, dtype=np.float32)
    Wkr = np.asarray(Wkr, dtype=np.float32)
    Wkvd = np.asarray(Wkvd, dtype=np.float32)
    Wku = np.asarray(Wku, dtype=np.float32)
    Wvu = np.asarray(Wvu, dtype=np.float32)
    Wo = np.asarray(Wo, dtype=np.float32)
    lobo_log = np.asarray(lobo_log, dtype=np.float32)

    if "nc" not in _CACHE:
        _CACHE["nc"] = _build_program()
    nc = _CACHE["nc"]

    cosq, sinq = _rope_tables()
    msk = _masks()
    in_maps = []
    for core in range(8):
        b, g = core // NG, core % NG
        hs = slice(g * HL * DH, (g + 1) * HL * DH)
        # packed q-projection weight blocks (128 cols each):
        #   0: [Wq_h0|Wq_h1]*S   1: [Wq_h2|Wq_h3]*S
        #   2: [Wqr_h0..h3]*S    3: [Wkr | zeros]
        wq_pack = np.zeros((E, HL * 128), dtype=np.float32)
        for h in range(HL):
            gh = g * HL + h
            blk, off = h // 2, (h % 2) * DH
            wq_pack[:, blk * 128 + off:blk * 128 + off + DH] = (
                Wq[:, gh * DH:(gh + 1) * DH] * SCALE)
            wq_pack[:, 256 + h * DR:256 + (h + 1) * DR] = (
                Wqr[:, gh * DR:(gh + 1) * DR] * SCALE)
        wq_pack[:, 384:384 + DR] = Wkr
        cv = np.broadcast_to(
            np.exp(lobo_log[g * HL:(g + 1) * HL])[None, :], (32, HL))
        in_maps.append({
            "xT": _bf(x[b].T),
            "wq": _bf(wq_pack),
            "wkvd": _bf(Wkvd),
            "wku": _bf(Wku[:, hs]),
            "wvu": _bf(Wvu[:, hs]),
            "wo": _bf(Wo[hs, :]),
            "cosq": _bf(cosq), "sinq": _bf(sinq),
            "cvals": np.ascontiguousarray(cv, dtype=np.float32),
            "masks": _bf(msk),
            "ident": _bf(np.eye(128, dtype=np.float32)),
        })

    trace = bool(os.environ.get("BASS_TRACE_KERNEL"))
    bkr = run_bass_kernel_spmd(
        nc, in_maps, core_ids=list(range(8)), trace=trace)
    if trace:
        print(f"HW exec time: {bkr.exec_time_ns} ns")
        if bkr.instructions_and_trace is not None:
            print("trace:", bkr.instructions_and_trace[1])
        _CACHE["last_result"] = bkr
    res = bkr.results
    out = np.zeros((B, T, E), dtype=np.float32)
    for core in range(8):
        out[core // NG] += res[core]["out"].astype(np.float32)
    return out


# revision 27
# speedup vs baseline: 1.8144x; 1.0130x over previous
"""MLA (multi-head latent attention) Bass kernel for 8 trn2 NeuronCores — v3.

Sharding: core = b*4 + g  (b in {0,1} batches, g in {0..3} head-groups of 4 heads).

Structure (v3): projections and attention are interleaved per query-group so
the tensor engine never idles long enough for the PE HAM clock-gate to drop
to 1.2 GHz:

  for tg in 0..3:
    proj(tg):  q_c / q_rope / k_rope / latent / k_c / V for columns tg
    for h in 0..3: attention stage (h, qg=tg)  [kc pairs, fused exp per pair]
    per-qg epilogue batched across heads (one [128,2048] comb transpose)
    normalization for qg-1 pops one stage behind; outproj(qg-1) streams

Key engine assignments (measured: ~170ns fixed cost per DVE op, PSUM reads
force 1x mode, gpsimd has no PSUM access and no max op):
  tensor: matmuls + additive causal masks (ident @ (-1e4 pattern) accumulated
          into the scores PSUM) + r-broadcast (K=1 ones matmul)
  scalar: exp (one [128,1024] ACT per kc pair) + PSUM->SBUF copies w/ shifts
  vector: row-max tracking (exp'd tiles), emax partition-reduce via 32x32
          transpose + shuffle tree in (j,b)-permuted layout, reciprocal via
          transpose round-trip, remaining PSUM copies
  gpsimd: SBUF-only rope combine ops, big memsets
"""

import math
import os

import numpy as np
import ml_dtypes

import concourse.bass as bass
import concourse.mybir as mybir
import concourse.tile as _tile_mod
from concourse.tile import TileContext
from concourse.vector_clock import ScopedClock, VectorClock
import bass_rust as _bass_rust
from concourse.bass_utils import run_bass_kernel_spmd

_N_PROCS = _bass_rust.N_PROCS
BF = ml_dtypes.bfloat16


def _split_drain_and_barrier(self, tick_clock, wait_clock):
    """Replacement for TileContext._drain_and_barrier: the stock version puts
    the whole global vector clock (up to 27 sem waits) on one Drain, which this
    walrus rejects ("Too many sync wait commands").  Emit one Drain per
    outstanding processor instead."""
    gc = tick_clock.global_clock
    procs = [p for p in range(_N_PROCS) if gc[p] > 0]
    for p in procs:
        vc = VectorClock([gc[q] if q == p else 0 for q in range(_N_PROCS)])
        d = self.nc.sync.drain()
        wait_clock.add_sem_waits(d.ins, ScopedClock({None: vc}))
    self.nc.all_engine_barrier()
    popped = self.nc._tile_sem_poison_stack.pop()
    assert popped is self._sem_poison
    self.nc.clear_and_free_semaphores(list(self.sems.allocated().values()))
    self.nc.all_engine_barrier()


_tile_mod.TileContext._drain_and_barrier = _split_drain_and_barrier

# ---------------------------------------------------------------------------
# This walrus build allows only ONE sync-wait per instruction ("Too many sync
# wait commands").  Post-process the BIR JSON: excess waits are hoisted onto
# same-engine NoOp carriers inserted immediately before the instruction (same
# program point on the engine's sequential stream -> semantics unchanged).
# NoOp (unlike Drain) does not flush the engine pipeline: ~13ns vs ~500ns.
# ---------------------------------------------------------------------------
_orig_to_json_bytes = bass.Bass.to_json_bytes
_WAIT_LIMITS = {"Drain": 1, "DMACopy": 1}
_DEF_WAIT_LIMIT = 1


def _to_json_split_waits(self, *a, **kw):
    import json as _json
    data = _json.loads(_orig_to_json_bytes(self, *a, **kw))
    nid = 0
    for f in data.get("functions", []):
        for bb in f.get("blocks", []):
            out = []
            for inst in bb.get("instructions", []):
                si = inst.get("sync_info")
                if isinstance(si, dict):
                    w = si.get("on_wait")
                    if isinstance(w, list):
                        k = _WAIT_LIMITS.get(inst.get("opcode"), _DEF_WAIT_LIMIT)
                        if len(w) > k:
                            extra, keep = w[:-k], w[-k:]
                            for wt in extra:
                                out.append({
                                    "debug": inst.get("debug"),
                                    "engine": inst["engine"],
                                    "ins": [], "outs": [],
                                    "name": f"wsplit-{nid}",
                                    "opcode": "NoOp",
                                    "sync_info": {"on_update": [],
                                                  "on_wait": [wt]},
                                })
                                nid += 1
                            si["on_wait"] = keep
                out.append(inst)
            bb["instructions"] = out
    return _json.dumps(data).encode()


bass.Bass.to_json_bytes = _to_json_split_waits

B, T, E = 2, 2048, 1024
H, DH = 16, 64
DKV = 256
DR = 32
HL = 4              # heads per core
NG = 4              # head groups
SCALE = 1.0 / math.sqrt(DH + DR)
TG = 512            # query-group width
KC = 128            # key-chunk width
NTG = T // TG       # 4
NKC = T // KC       # 16
EC = E // 128       # 8  e-chunks
CC = DKV // 128     # 2  latent chunks

F32 = mybir.dt.float32
BF16 = mybir.dt.bfloat16
AF = mybir.ActivationFunctionType
ALU = mybir.AluOpType
AX = mybir.AxisListType

_CACHE = {}

_ROT16 = [(i + 16) % 32 for i in range(32)]
_IDENT = list(range(32))


def _build_program():
    nc = bass.Bass()

    xT = nc.declare_dram_parameter("xT", [E, T], BF16, isOutput=False)
    wq = nc.declare_dram_parameter("wq", [E, HL * 128], BF16, isOutput=False)
    wkvd = nc.declare_dram_parameter("wkvd", [E, DKV], BF16, isOutput=False)
    wku = nc.declare_dram_parameter("wku", [DKV, HL * DH], BF16, isOutput=False)
    wvu = nc.declare_dram_parameter("wvu", [DKV, HL * DH], BF16, isOutput=False)
    wo = nc.declare_dram_parameter("wo", [HL * DH, E], BF16, isOutput=False)
    cosq = nc.declare_dram_parameter("cosq", [128, T], BF16, isOutput=False)
    sinq = nc.declare_dram_parameter("sinq", [128, T], BF16, isOutput=False)
    cbc = nc.declare_dram_parameter("cbc", [32, HL * 16], F32, isOutput=False)
    masks = nc.declare_dram_parameter("masks", [128, 4 * TG], BF16, isOutput=False)
    ident = nc.declare_dram_parameter("ident", [128, 128], BF16, isOutput=False)
    out = nc.declare_dram_parameter("out", [T, E], BF16, isOutput=True)

    xTr = xT.rearrange("(c p) t -> p c t", p=128)

    with TileContext(nc) as tc:
        from contextlib import ExitStack

        with ExitStack() as ctx:
            singles = ctx.enter_context(tc.tile_pool(name="singles", bufs=1))
            pool = ctx.enter_context(tc.tile_pool(name="pool", bufs=2))
            psp = ctx.enter_context(tc.tile_pool(name="psp", bufs=1, space="PSUM"))

            # ---------------- weights + tables (all bf16 from host) --------
            wq_sb = singles.tile([128, EC, HL * 128], BF16)
            nc.sync.dma_start(out=wq_sb, in_=wq.rearrange("(c p) f -> p c f", p=128))
            wkvd_sb = singles.tile([128, EC, DKV], BF16)
            nc.sync.dma_start(
                out=wkvd_sb, in_=wkvd.rearrange("(c p) f -> p c f", p=128))
            cos_sb = singles.tile([128, T], BF16)
            nc.sync.dma_start(out=cos_sb, in_=cosq[:, :])
            sin_sb = singles.tile([128, T], BF16)
            nc.sync.dma_start(out=sin_sb, in_=sinq[:, :])
            wku_sb = singles.tile([128, CC, HL * DH], BF16)
            nc.sync.dma_start(out=wku_sb, in_=wku.rearrange("(c p) f -> p c f", p=128))
            wvu_sb = singles.tile([128, CC, HL * DH], BF16)
            nc.sync.dma_start(out=wvu_sb, in_=wvu.rearrange("(c p) f -> p c f", p=128))
            masks_sb = singles.tile([128, 4, TG], BF16)
            nc.sync.dma_start(
                out=masks_sb, in_=masks.rearrange("p (j y) -> p j y", j=4))
            ident_sb = singles.tile([128, 128], BF16)
            nc.sync.dma_start(out=ident_sb, in_=ident[:, :])

            # x chunks: pool ring, 2 query-groups in flight
            xt_tiles = {}

            def load_xt(tg):
                ts_ = slice(tg * TG, (tg + 1) * TG)
                for ec_ in range(EC):
                    t_ = pool.tile([128, TG], BF16, name=f"xt{tg}_{ec_}",
                                   tag="xt", bufs=16)
                    nc.sync.dma_start(out=t_, in_=xTr[:, ec_, ts_])
                    xt_tiles[(tg, ec_)] = t_

            load_xt(0)
            wo_sb = singles.tile([128, 2, E], BF16)
            nc.sync.dma_start(out=wo_sb, in_=wo.rearrange("(c p) e -> p c e", p=128))
            cbc_sb = singles.tile([32, HL * 16], F32)
            nc.sync.dma_start(out=cbc_sb, in_=cbc[:, :])
            load_xt(1)

            ones1 = singles.tile([1, DH], BF16)
            nc.vector.memset(ones1, 1.0)
            dext = singles.tile([32, HL * TG], F32)
            nc.gpsimd.memset(dext, 1.0)
            rstage = singles.tile([32, HL * TG], BF16)
            nc.gpsimd.memset(rstage, 1.0)

            # ---------------- persistent activation tiles ----------------
            latT_sb = singles.tile([128, CC, T], BF16)
            qT = [singles.tile([96, T], BF16, name=f"qT{h}") for h in range(HL)]
            kT = [singles.tile([96, T], BF16, name=f"kT{h}") for h in range(HL)]
            v_sb = singles.tile([128, NKC, HL, DH + 1], BF16)
            nc.gpsimd.memset(v_sb, 1.0)
            yraw_sb = singles.tile([DH, HL, T], BF16)
            yT_sb = singles.tile([128, 2, T], BF16)

            norm_queue = []

            def drain_norms(n):
                for _ in range(n):
                    if norm_queue:
                        norm_queue.pop(0)()

            def emit_norm(h, qg, rback_ap):
                qs = slice(qg * TG, (qg + 1) * TG)
                bcp = psp.tile([DH, TG], F32, name="bc", tag="B", bufs=2)
                nc.tensor.matmul(bcp, ones1, rback_ap)
                nc.vector.tensor_mul(
                    yT_sb[(h % 2) * DH:(h % 2 + 1) * DH, h // 2, qs],
                    yraw_sb[0:DH, h, qs], bcp)

            def emit_outproj(qg):
                for tt in range(4 * qg, 4 * qg + 4):
                    op = psp.tile([128, E], F32, name="op", tag="S", bufs=2)
                    for eg in range(2):
                        for fc in range(2):
                            nc.tensor.matmul(
                                op[:, eg * TG:(eg + 1) * TG],
                                yT_sb[:, fc, tt * 128:(tt + 1) * 128],
                                wo_sb[:, fc, eg * TG:(eg + 1) * TG],
                                start=(fc == 0), stop=(fc == 1))
                    ost = pool.tile([128, E], BF16, name="ost", tag="ost", bufs=2)
                    if tt % 2 == 0:
                        nc.vector.tensor_copy(ost, op)
                    else:
                        nc.scalar.copy(ost, op)
                    nc.sync.dma_start(
                        out=out[tt * 128:(tt + 1) * 128, :], in_=ost)

            # =======================================================
            for tg in range(NTG):
                ts = slice(tg * TG, (tg + 1) * TG)
                if tg + 2 < NTG:
                    load_xt(tg + 2)
                xts = [xt_tiles[(tg, ec)] for ec in range(EC)]

                # ---- projections for this query-group ----
                def proj_pair(blk0, blk1, name):
                    sp = psp.tile([128, 2 * TG], F32, name=name, tag="S", bufs=2)
                    for hv, blk in ((0, blk0), (1, blk1)):
                        fs = slice(blk * 128, (blk + 1) * 128)
                        for ec in range(EC):
                            nc.tensor.matmul(
                                sp[:, hv * TG:(hv + 1) * TG],
                                wq_sb[:, ec, fs], xts[ec],
                                start=(ec == 0), stop=(ec == EC - 1))
                    return sp

                qcp = proj_pair(0, 1, "qc")    # [qc h0|h1 , qc h2|h3]
                qrp = proj_pair(2, 3, "qr")    # [qr 4x32 , kr|junk]
                lp = psp.tile([128, 2 * TG], F32, name="lat", tag="S", bufs=2)
                for cc in range(CC):
                    for ec in range(EC):
                        nc.tensor.matmul(
                            lp[:, cc * TG:(cc + 1) * TG],
                            wkvd_sb[:, ec, cc * 128:(cc + 1) * 128], xts[ec],
                            start=(ec == 0), stop=(ec == EC - 1))

                # q_c: even heads DVE (equal-base PSUM copy), odd heads scalar
                nc.vector.tensor_copy(qT[0][0:DH, ts], qcp[0:DH, 0:TG])
                nc.scalar.copy(qT[1][0:DH, ts], qcp[DH:128, 0:TG])
                nc.vector.tensor_copy(qT[2][0:DH, ts], qcp[0:DH, TG:2 * TG])
                nc.scalar.copy(qT[3][0:DH, ts], qcp[DH:128, TG:2 * TG])

                # q rope, all 4 heads batched; SBUF-only combine ops on gpsimd
                sw = pool.tile([128, TG], F32, name="sw", tag="qsw", bufs=2)
                nc.vector.stream_shuffle(sw, qrp[:, 0:TG], _ROT16)
                t1 = pool.tile([128, TG], BF16, name="t1", tag="t1", bufs=2)
                nc.vector.tensor_mul(t1, qrp[:, 0:TG], cos_sb[:, ts])
                t2 = pool.tile([128, TG], BF16, name="t2", tag="t2", bufs=2)
                nc.gpsimd.tensor_mul(t2, sw, sin_sb[:, ts])
                rq = pool.tile([128, TG], BF16, name="rq", tag="rq", bufs=2)
                nc.gpsimd.tensor_add(rq, t1, t2)
                for h in range(HL):
                    nc.scalar.copy(qT[h][DH:96, ts], rq[h * DR:(h + 1) * DR, :])
                # k rope (rows 0:32 of the second half)
                sw2 = pool.tile([DR, TG], F32, name="sw2", tag="sw2", bufs=2)
                nc.vector.stream_shuffle(sw2, qrp[0:DR, TG:2 * TG], _ROT16)
                k1 = pool.tile([DR, TG], BF16, name="k1", tag="k1", bufs=2)
                nc.vector.tensor_mul(k1, qrp[0:DR, TG:2 * TG], cos_sb[0:DR, ts])
                k2 = pool.tile([DR, TG], BF16, name="k2", tag="k2", bufs=2)
                nc.gpsimd.tensor_mul(k2, sw2, sin_sb[0:DR, ts])
                nc.vector.tensor_add(kT[0][DH:96, ts], k1, k2)
                for h in range(1, HL):
                    nc.scalar.copy(kT[h][DH:96, ts], kT[0][DH:96, ts])
                # latent -> SBUF bf16
                nc.vector.tensor_copy(latT_sb[:, 0, ts], lp[:, 0:TG])
                nc.vector.tensor_copy(latT_sb[:, 1, ts], lp[:, TG:2 * TG])

                # k_c = Wku^T latent (per head-pair)
                for hp in range(2):
                    mp = psp.tile([128, TG], F32, name="kc", tag="M", bufs=2)
                    for cc in range(CC):
                        nc.tensor.matmul(
                            mp, wku_sb[:, cc, hp * 128:(hp + 1) * 128],
                            latT_sb[:, cc, ts],
                            start=(cc == 0), stop=(cc == CC - 1))
                    nc.vector.tensor_copy(kT[2 * hp][0:DH, ts], mp[0:DH, :])
                    nc.scalar.copy(kT[2 * hp + 1][0:DH, ts], mp[DH:128, :])

                # V natural layout
                for tt in range(4 * tg, 4 * tg + 4):
                    bp = psp.tile([128, HL * DH], F32, name="v", tag="B", bufs=2)
                    for cc in range(CC):
                        nc.tensor.matmul(
                            bp, latT_sb[:, cc, tt * 128:(tt + 1) * 128],
                            wvu_sb[:, cc, :],
                            start=(cc == 0), stop=(cc == CC - 1))
                    nc.vector.tensor_copy(v_sb[:, tt, :, 0:DH], bp)

                # ---- attention stages (h, qg=tg) ----
                qg = tg
                qs = ts
                nkc = 4 * qg + 4
                npair = nkc // 2
                diag = 4 * qg
                # per-qg comb, all 4 heads side by side
                comb = pool.tile([128, HL, TG], BF16, name="comb", tag="comb",
                                 bufs=2)
                for h in range(HL):
                    yps = psp.tile([DH + 1, TG], F32, name="py", tag="M", bufs=2)
                    for pr in range(npair):
                        kca, kcb = 2 * pr, 2 * pr + 1
                        sp = psp.tile([128, 2 * TG], F32, name="ps", tag="S",
                                      bufs=2)
                        for hv, kc in ((0, kca), (1, kcb)):
                            half = sp[:, hv * TG:(hv + 1) * TG]
                            j = kc - diag
                            nc.tensor.matmul(
                                half, kT[h][:, kc * KC:(kc + 1) * KC],
                                qT[h][:, qs],
                                start=True, stop=(j < 0))
                            if j >= 0:
                                # additive causal mask via ident matmul
                                w = 128 * j + 128
                                nc.tensor.matmul(
                                    sp[:, hv * TG:hv * TG + w],
                                    ident_sb, masks_sb[:, j, 0:w],
                                    start=False, stop=True)
                        pt = pool.tile([128, 2 * TG], BF16, name="pt", tag="pt",
                                       bufs=3)
                        nc.scalar.activation(pt, sp, AF.Exp)
                        if pr == 0:
                            nc.vector.tensor_max(
                                comb[:, h, :], pt[:, 0:TG], pt[:, TG:2 * TG])
                        else:
                            nc.vector.tensor_max(
                                comb[:, h, :], comb[:, h, :], pt[:, 0:TG])
                            nc.vector.tensor_max(
                                comb[:, h, :], comb[:, h, :], pt[:, TG:2 * TG])
                        nc.tensor.matmul(
                            yps, v_sb[:, kca, h, :], pt[:, 0:TG],
                            start=(pr == 0), stop=False)
                        nc.tensor.matmul(
                            yps, v_sb[:, kcb, h, :], pt[:, TG:2 * TG],
                            start=False, stop=(pr == npair - 1))
                    nc.vector.tensor_copy(yraw_sb[:, h, qs], yps[0:DH, :])
                    # dsum (f32) into the h-major d row
                    nc.scalar.copy(dext[0:1, h * TG:(h + 1) * TG],
                                   yps[DH:DH + 1, :])
                    drain_norms(2 if tg == NTG - 1 else 1)

                # ---- per-qg epilogue batched over heads ----
                combT = pool.tile([128, HL * TG], BF16, name="combT",
                                  tag="combT", bufs=1)
                nc.vector.transpose(combT, comb.rearrange("p h y -> p (h y)"))
                red = pool.tile([128, HL * 16], F32, name="red", tag="red",
                                bufs=2)
                nc.vector.reduce_max(
                    red, combT.rearrange("p (b j) -> p b j", j=32), axis=AX.X)
                s1 = pool.tile([32, HL * 16], F32, name="s1", tag="s1", bufs=2)
                nc.vector.stream_shuffle(s1, red[32:64, :], _IDENT)
                s2 = pool.tile([32, HL * 16], F32, name="s2", tag="s2", bufs=2)
                nc.vector.stream_shuffle(s2, red[64:96, :], _IDENT)
                s3 = pool.tile([32, HL * 16], F32, name="s3", tag="s3", bufs=2)
                nc.vector.stream_shuffle(s3, red[96:128, :], _IDENT)
                nc.vector.tensor_max(s1, red[0:32, :], s1)
                nc.vector.tensor_max(s2, s2, s3)
                emfP = pool.tile([32, HL * 16], F32, name="emf", tag="emf",
                                 bufs=2)
                nc.vector.tensor_max(emfP, s1, s2)
                # d = dsum + C_h*emax in perm layout; C broadcast tile from host
                nc.vector.tensor_mul(emfP, emfP, cbc_sb)
                dTt = pool.tile([32, HL * TG], F32, name="dTt", tag="dTt",
                                bufs=1)
                nc.vector.transpose(dTt, dext)
                dP = pool.tile([32, HL * 16], F32, name="dP", tag="dP", bufs=2)
                nc.vector.tensor_add(
                    dP, emfP,
                    dTt.rearrange("p (b j) -> p b j", j=32)[:, :, 0])
                with nc.allow_low_precision(reason="r in bf16; tol 2e-2"):
                    nc.vector.reciprocal(
                        rstage.rearrange("p (b j) -> p b j", j=32)[:, :, 0], dP)
                rback = pool.tile([32, HL * TG], BF16, name="rbk", tag="rbk",
                                  bufs=2)
                nc.vector.transpose(rback, rstage)
                for h in range(HL):
                    norm_queue.append(
                        (lambda hh=h, qq=qg,
                         r_=rback[0:1, h * TG:(h + 1) * TG]:
                         emit_norm(hh, qq, r_)))
                if qg > 0:
                    norm_queue.append(lambda qq=qg - 1: emit_outproj(qq))
            # tail
            drain_norms(len(norm_queue))
            emit_outproj(NTG - 1)

    return nc


def _masks():
    """Additive causal masks: -1e4 on masked (k > q) cells, 0 elsewhere."""
    x = np.arange(128)[:, None]
    y = np.arange(TG)[None, :]
    ms = [np.where(x - y + 128 * j <= 0, 0.0, -1e4).astype(np.float32)
          for j in range(4)]
    return np.concatenate(ms, axis=1)  # [128, 4*TG]


def _rope_tables():
    half = DR // 2
    inv = 1.0 / (10000.0 ** (np.arange(half, dtype=np.float64) / half))
    ang = np.arange(T, dtype=np.float64)[:, None] * inv[None, :]  # (T, half)
    cos = np.cos(ang).T  # (half, T)
    sin = np.sin(ang).T
    cosk = np.concatenate([cos, cos], axis=0)                 # (32, T)
    sink = np.concatenate([-sin, sin], axis=0)
    cosq = np.tile(cosk, (HL, 1))                             # (128, T)
    sinq = np.tile(sink, (HL, 1))
    return cosq.astype(np.float32), sinq.astype(np.float32)


def _bf(x):
    return np.ascontiguousarray(np.asarray(x, dtype=np.float32).astype(BF))


def kernel(x, Wq, Wqr, Wkr, Wkvd, Wku, Wvu, Wo, lobo_log):
    x = np.asarray(x, dtype=np.float32)
    Wq = np.asarray(Wq, dtype=np.float32)
    Wqr = np.asarray(Wqr, dtype=np.float32)
    Wkr = np.asarray(Wkr, dtype=np.float32)
    Wkvd = np.asarray(Wkvd, dtype=np.float32)
    Wku = np.asarray(Wku, dtype=np.float32)
    Wvu = np.asarray(Wvu, dtype=np.float32)
    Wo = np.asarray(Wo, dtype=np.float32)
    lobo_log = np.asarray(lobo_log, dtype=np.float32)

    if "nc" not in _CACHE:
        _CACHE["nc"] = _build_program()
    nc = _CACHE["nc"]

    cosq, sinq = _rope_tables()
    msk = _masks()
    in_maps = []
    for core in range(8):
        b, g = core // NG, core % NG
        hs = slice(g * HL * DH, (g + 1) * HL * DH)
        # packed q-projection weight blocks (128 cols each):
        #   0: [Wq_h0|Wq_h1]*S   1: [Wq_h2|Wq_h3]*S
        #   2: [Wqr_h0..h3]*S    3: [Wkr | zeros]
        wq_pack = np.zeros((E, HL * 128), dtype=np.float32)
        for h in range(HL):
            gh = g * HL + h
            blk, off = h // 2, (h % 2) * DH
            wq_pack[:, blk * 128 + off:blk * 128 + off + DH] = (
                Wq[:, gh * DH:(gh + 1) * DH] * SCALE)
            wq_pack[:, 256 + h * DR:256 + (h + 1) * DR] = (
                Wqr[:, gh * DR:(gh + 1) * DR] * SCALE)
        wq_pack[:, 384:384 + DR] = Wkr
        # C_h broadcast tile matching the (j, h*16+b) permuted layout
        cv = np.exp(lobo_log[g * HL:(g + 1) * HL])
        cbcv = np.broadcast_to(np.repeat(cv, 16)[None, :], (32, HL * 16))
        in_maps.append({
            "xT": _bf(x[b].T),
            "wq": _bf(wq_pack),
            "wkvd": _bf(Wkvd),
            "wku": _bf(Wku[:, hs]),
            "wvu": _bf(Wvu[:, hs]),
            "wo": _bf(Wo[hs, :]),
            "cosq": _bf(cosq), "sinq": _bf(sinq),
            "cbc": np.ascontiguousarray(cbcv, dtype=np.float32),
            "masks": _bf(msk),
            "ident": _bf(np.eye(128, dtype=np.float32)),
        })

    trace = bool(os.environ.get("BASS_TRACE_KERNEL"))
    bkr = run_bass_kernel_spmd(
        nc, in_maps, core_ids=list(range(8)), trace=trace)
    if trace:
        print(f"HW exec time: {bkr.exec_time_ns} ns")
        if bkr.instructions_and_trace is not None:
            print("trace:", bkr.instructions_and_trace[1])
        _CACHE["last_result"] = bkr
    res = bkr.results
    out = np.zeros((B, T, E), dtype=np.float32)
    for core in range(8):
        out[core // NG] += res[core]["out"].astype(np.float32)
    return out
